# revision 13
# baseline (speedup 1.0000x reference)
"""Multi-head attention (S=4096, D=1024, H=16) on 8 trn2 NeuronCores.

Sharding: 2 heads per core (tensor-parallel on Q/K/V column splits and
dense row split). Each core computes a partial [S, D] output; host sums
the 8 partials (the unshard step for row-parallel TP).

Per-core design (bf16 operands, fp32 PSUM accumulate):
  The ACT engine's exp over the S*S*2 logits (256 x [128,1024] tiles,
  ~1.04us each) is the hard floor (~266us busy); everything else is
  scheduled to hide inside it (this build: ~292us total = 10.8us
  startup + 266us exp + ~2.5us exp stalls + ~12.4us finalize tail).
  - logits: lt[128t, 2h*512s] = k-block^T q-window per head (2 matmuls,
    512 rows each).
  - PV streams v, not P: stationary = pt s-block [128t,128s], moving =
    v_aug[128t,65] -> acc[128s,65] accumulated over 32 t-blocks (65
    rows/matmul instead of 512); column 64 of v_aug is ones so the
    softmax denominator lands in acc[...,64] already laid out per
    s-partition. One accumulation group per PSUM 2KB zero-region: only
    (tb0,k0) starts and (tb31,k3) stops each head's bank.
  - normalization: reciprocal of the denominator, then per-partition
    tensor_scalar_mul fused into the acc->SBUF copy.
  - v-projection emits v directly in [s-part, dk] layout (stationary =
    x-block, moving = Wv chunk): no PE transposes of V.
  - output projection: per s-block, transpose normalized attn
    ([128s,64]->[64dk,128s] via PE, 1 cyc/row in bf16), then
    yp[128s,512d] = attnT^T wo; y is bf16, host sums partials in fp32.
  PSUM (8 banks): lt 2x2 (ping-pong), acc 2 (padded [128,2,4,128],
  single-buffered across waves), 2-slot scratch ring shared by proj
  psum / transposes / yp; the last wave's yp also rotates through the
  then-dead lt+acc banks so the tail pipelines 3-deep.
  Schedule: one exp per iteration paces everything. Between consecutive
  lt emissions the PE tolerates only ~1.2us of other work, so PV /
  finalize thunks pop from a floor-gated list under a PE-ns budget;
  projection deadlines are hard (logits read qT/kT at fixed iterations)
  and ordering gates enforce pv(w) < scales(w) < pv(w+1) (acc handoff).
  GPSIMD cannot touch PSUM, so all PSUM evacuation is DVE/ACT.
  Startup: matmul cost is priced at decode time ~36 instructions ahead
  of execution, so 8 big + 36 tiny memset-fed warm-up matmuls both ramp
  the PE p-state and flush the decode queue before the first real
  projections; DMA order puts wk/wq/x-piece-0 first (HWDGE serializes
  at ~625ns per dma_start) and weights are host-re-laid so every DMA
  descriptor is a contiguous 2KB run.
"""

import numpy as np
from collections import deque
from contextlib import ExitStack

S = 4096
D = 1024
NCORES = 8
HD = 128  # head-dim span per core (2 heads x 64)
DK = 64
NT = S // 128   # 32 t-blocks
NWV = 8         # s-waves of 512
LAG_W0 = 24   # wave-0 PV lag: pushes PV+v work past the k/q-proj burst
LAG = 2       # steady-state PV lag (lt(i+1) is emitted before the
              # pops, so a briefly-waiting PV never blocks logits)
LAG_LAST = 2  # final wave drains tight to shorten the tail
PE_BUDGET = 600   # max popped PE-ns per iteration (guards the exp cadence)
V_TRACK = True    # v-blocks emitted from inside wave-0 PV pops (+4 lookahead)
Q_MID = False     # q pieces at wave end (mid-wave collides with finalize)
LAG_W1 = 14       # wave-1 PV lag (wave-0 backlog still draining)

# Schraudolph exp-on-DVE: i16 = trunc/round(A*(0.125*lt) + B); bitcast bf16
# gives e^z * (1 + O(3%)).  A = 128*log2(e) folded with the 0.125 logit
# scale; B centers the linear-in-mantissa interpolation error (calibrated
# against the real convert rounding via test.py sweep).
SCH_A = float(128.0 / np.log(2.0) * 0.125)
SCH_B = 16256.0 - 5.0
# t-blocks whose exp runs on DVE (Schraudolph) instead of ACT, per wave.
# 12/32 offload in an A,A,D-ish weave: ACT does 20 exps/wave (~21us), DVE
# ~14us + evacuations; both below the PE wave time (~27.8us), making PE the
# pacing engine.  tb 0-3 stay on ACT so the wave-boundary recip/scale (the
# serial acc handoff, accpool bufs=1) isn't queued behind a 1.2us DVE exp.
DVE_TBS = {4, 6, 9, 11, 14, 16, 19, 21, 24, 26, 29, 31}
DVE_TBS_W0 = {14, 17, 20, 23, 26, 29}  # wave 0: DVE busy with v/proj backlog

_NC_CACHE = {}


def _split_multi_waits(nc, mybir):
    """This walrus build encodes at most ~2 sync commands per instruction
    (1 for matmul/drain). Keep <=1 wait on every compute/DMA instruction and
    move the rest into standalone dual-condition EventSemaphore instructions
    inserted immediately before it on the same engine (same wait point, so
    semantics are unchanged)."""
    n = 0
    used = set()
    for b in nc.m.functions[0].blocks:
        for inst in b.instructions:
            si = inst.sync_info
            if si:
                for w in (si.on_wait or []):
                    used.add(w.id)
                for u in (si.on_update or []):
                    used.add(u.id)
    free_ids = [i for i in range(max(used) + 1, max(used) + 32)]
    sems = {}

    def eng_sem(eng):
        if eng not in sems:
            sems[eng] = (free_ids.pop(0), f"wsplit_{len(sems)}")
        return sems[eng]

    for b in nc.m.functions[0].blocks:
        il = b.instructions
        new = []
        for inst in il:
            si = inst.sync_info
            waits = list(si.on_wait) if si and si.on_wait else []
            upds = list(si.on_update) if si and si.on_update else []
            if type(inst).__name__ == "InstEventSemaphore":
                new.append(inst)
                continue
            if len(waits) > 1:
                excess, keep = waits[:-1], waits[-1:]
                for i in range(0, len(excess), 2):
                    sid, sname = eng_sem(inst.engine)
                    ev = mybir.InstEventSemaphore(
                        name=f"{inst.name}_ws{i}", engine=inst.engine,
                        ins=[], outs=[],
                        sync_info=mybir.SyncInfo(
                            on_wait=excess[i:i + 2],
                            on_update=[mybir.SyncUpdate(
                                sync_type="semaphore", id=sid,
                                ant_name=sname, update_mode="sem-inc",
                                update_value=1, update_reg=None)]))
                    new.append(ev)
                    n += 1
                inst.sync_info = mybir.SyncInfo(on_wait=keep, on_update=upds)
            new.append(inst)
        il[:] = new
    return n


def _build_nc():
    import concourse.bass as bass
    import concourse.tile as tile
    import concourse.mybir as mybir

    fp32 = mybir.dt.float32
    bf16 = mybir.dt.bfloat16
    i16 = mybir.dt.int16
    AF = mybir.ActivationFunctionType
    ALU = mybir.AluOpType

    nc = bass.Bass()
    xT = nc.dram_tensor("xT", [D, S], bf16, kind="ExternalInput")
    # weights pre-laid host-side as [p, c, m] so each DMA descriptor is a
    # contiguous 2KB run (256B descriptors pay a 2x latency penalty)
    wqT = nc.dram_tensor("wqT", [128, 8 * HD], bf16, kind="ExternalInput")
    wkT = nc.dram_tensor("wkT", [128, 8 * HD], bf16, kind="ExternalInput")
    wvT = nc.dram_tensor("wvT", [128, 8 * HD], bf16, kind="ExternalInput")
    woT = nc.dram_tensor("woT", [HD, D], bf16, kind="ExternalInput")
    ident = nc.dram_tensor("ident", [128, 128], bf16, kind="ExternalInput")
    y = nc.dram_tensor("y", [S, D], bf16, kind="ExternalOutput")

    with tile.TileContext(nc) as tc, ExitStack() as ctx, \
         nc.allow_low_precision(reason="bf16 operands within rel-err budget"):
        sb = ctx.enter_context(tc.tile_pool(name="sb", bufs=1))
        qT_sb = sb.tile([128, S], bf16, tag="qT")
        kT_sb = sb.tile([128, S], bf16, tag="kT")
        # v_aug[:, tb, h, 0:64] = v block for head h; [..., 64] = ones
        v_aug = sb.tile([128, NT, 2, DK + 1], bf16, tag="vaug")
        wq_sb = sb.tile([128, 8, HD], bf16, tag="wq")
        wk_sb = sb.tile([128, 8, HD], bf16, tag="wk")
        wv_sb = sb.tile([128, 8, HD], bf16, tag="wv")
        wo_sb = sb.tile([HD, D], bf16, tag="wo")
        id_sb = sb.tile([128, 128], bf16, tag="id")
        warm = sb.tile([1, DK], fp32, tag="warm")
        wsrc = sb.tile([128, 512], bf16, tag="wsrc")

        xpool = ctx.enter_context(tc.tile_pool(name="xpool", bufs=8))
        ptpool = ctx.enter_context(tc.tile_pool(name="ptpool", bufs=LAG_W0 + 14))
        attnpool = ctx.enter_context(tc.tile_pool(name="attnpool", bufs=2))
        atTpool = ctx.enter_context(tc.tile_pool(name="atTpool", bufs=4))
        yopool = ctx.enter_context(tc.tile_pool(name="yopool", bufs=8))
        dpool = ctx.enter_context(tc.tile_pool(name="dpool", bufs=2))
        ltpool = ctx.enter_context(
            tc.tile_pool(name="ltpool", bufs=2, space="PSUM"))
        accpool = ctx.enter_context(
            tc.tile_pool(name="accpool", bufs=1, space="PSUM"))
        pspool = ctx.enter_context(
            tc.tile_pool(name="pspool", bufs=2, space="PSUM"))

        # ---- startup DMAs: critical path (wk, wq, x piece 0) first ----
        nc.sync.dma_start(wk_sb[:], wkT.rearrange("p (c m) -> p c m", c=8))
        nc.sync.dma_start(wq_sb[:], wqT.rearrange("p (c m) -> p c m", c=8))
        xTr = xT.rearrange("(c p) s -> p c s", p=128)
        xqs = {}

        def load_xq(p):
            xq = xpool.tile([128, 8, 512], bf16, tag="xq", name=f"xq_{p}")
            for c in range(0, 8, 2):
                nc.sync.dma_start(xq[:, c:c + 2, :],
                                  xTr[:, c:c + 2, p * 512:(p + 1) * 512])
            xqs[p] = xq

        load_xq(0)
        nc.sync.dma_start(wv_sb[:], wvT.rearrange("p (c m) -> p c m", c=8))
        nc.sync.dma_start(wo_sb[:], woT[:])
        nc.sync.dma_start(id_sb[:], ident[:])
        nc.vector.memset(v_aug[:, :, :, DK], 1.0)
        # Warm-up fed by an on-chip memset (no DMA dependency). Matmul cost
        # is priced at DECODE time, which runs ~36 instructions (exec-queue
        # depth) ahead of execution: to get the real projection matmuls
        # priced at the full-speed p-state, the PE must (a) be continuously
        # busy >3us and (b) have >=36 instructions in flight before them.
        # 8 big warms ramp the clock, 36 tiny ones flush the decode queue.
        nc.vector.memset(wsrc[:], 0.5)
        nc.scalar.activation(warm[:], wsrc[0:1, 0:DK], AF.Exp, scale=0.125)
        warm_slots = [
            pspool.tile([128, 512], fp32, tag="ps", name=f"warmps_{i}")
            for i in range(2)]
        warm_lt = [
            ltpool.tile([128, 1024], fp32, tag="lt", name=f"warmlt_{i}")
            for i in range(2)]
        for i in range(8):
            wp = (warm_slots[i % 2][:] if i % 4 < 2
                  else warm_lt[i % 2][:, 0:512])
            nc.tensor.matmul(wp, wsrc[:, 0:128], wsrc[:],
                             start=True, stop=True)
        for i in range(36):
            wp = (warm_slots[i % 2][:, 0:32] if i % 4 < 2
                  else warm_lt[i % 2][:, 0:32])
            nc.tensor.matmul(wp, wsrc[:, 0:128], wsrc[:, 0:32],
                             start=True, stop=True)
        # remaining x pieces; all 8 stay resident (q-proj of piece p runs
        # ~4 waves after its k-proj, so slots can't rotate)
        for p in range(1, 8):
            load_xq(p)

        # ---- projection pieces (as resumable quarter thunks) ----
        def qk_proj_thunks(p, w_sb, dst):
            """Four thunks of 2 chunks each for dst[:, p*512:(p+1)*512]."""
            hold = {}

            def quarter(i):
                def run():
                    if i == 0:
                        hold["ps"] = pspool.tile(
                            [128, 512], fp32, tag="ps",
                            name=f"pqk_{dst.tensor.name}_{p}")
                    ps = hold["ps"]
                    for c in range(2 * i, 2 * i + 2):
                        nc.tensor.matmul(ps[:], w_sb[:, c, :],
                                         xqs[p][:, c, :], start=(c == 0),
                                         stop=(c == 7))
                    if i == 3:
                        nc.vector.tensor_copy(
                            dst[:, p * 512:(p + 1) * 512], hold.pop("ps")[:])
                return run
            return [quarter(i) for i in range(4)]

        def v_block_thunk(tb):
            def run():
                p, i = tb // 4, tb % 4
                vps = pspool.tile([128, 128], fp32, tag="ps",
                                  name=f"vps_{tb}")
                for c in range(8):
                    nc.tensor.matmul(
                        vps[:], xqs[p][:, c, i * 128:(i + 1) * 128],
                        wv_sb[:, c, :], start=(c == 0), stop=(c == 7))
                nc.vector.tensor_copy(
                    v_aug[:, tb, :, 0:DK],
                    vps[:].rearrange("p (h d) -> p h d", h=2))
            return run

        # ---- startup projections: piece 0 of q and k, interleaved per
        # x-chunk so matmuls start as soon as each chunk DMA lands; the
        # two PSUM->SBUF copies run on different engines in parallel ----
        pq0 = pspool.tile([128, 512], fp32, tag="ps", name="pq0")
        pk0 = pspool.tile([128, 512], fp32, tag="ps", name="pk0")
        xq0 = xqs[0]
        for c in range(8):
            nc.tensor.matmul(pq0[:], wq_sb[:, c, :], xq0[:, c, :],
                             start=(c == 0), stop=(c == 7))
            nc.tensor.matmul(pk0[:], wk_sb[:, c, :], xq0[:, c, :],
                             start=(c == 0), stop=(c == 7))
        nc.scalar.copy(qT_sb[:, 0:512], pq0[:])
        # lt(0,0) needs only k block 0: give it its own small copy
        nc.vector.tensor_copy(kT_sb[:, 0:128], pk0[:, 0:128])
        nc.vector.tensor_copy(kT_sb[:, 128:512], pk0[:, 128:512])

        # deadline-ordered weave of the remaining projection work; each
        # entry is [deadline, pe_cost_ns, thunk]
        proj_q = deque()
        for p in range(1, 8):
            for j, t in enumerate(qk_proj_thunks(p, wk_sb, kT_sb)):
                proj_q.append([4 * p - 5 + j, 427, t])
        # v blocks 0..3 by deadline; the rest are emitted from inside the
        # wave-0 PV pops with a +4 lookahead, auto-tracking the drain rate
        v_done = set()

        def ensure_v(tb):
            if tb < NT and tb not in v_done:
                v_done.add(tb)
                v_block_thunk(tb)()

        n_pre_v = 4 if V_TRACK else NT
        for tb in range(n_pre_v):
            proj_q.append([LAG_W0 - 6 + tb, 427, lambda tb=tb: ensure_v(tb)])
        # q piece w must be fully copied before lt(w, 0) is emitted at
        # gi 32w-1: the last quarter lands at base+6 <= 32w-3
        for j, t in enumerate(qk_proj_thunks(1, wq_sb, qT_sb)):
            proj_q.append([(16 if Q_MID else 22) + 2 * j, 427, t])
        for w in range(2, NWV):
            for j, t in enumerate(qk_proj_thunks(w, wq_sb, qT_sb)):
                base = 32 * (w - 1) + (10 if Q_MID else 22)
                proj_q.append([base + 2 * j, 427, t])
        proj_q = deque(sorted(proj_q, key=lambda e: e[0]))

        # ---- attention machinery ----
        # pending: [floor, cost, kind, wave, thunk]; kind "pv" | "sc" | "fin".
        # Ordering gates (enforced at pop time, not just via floors):
        #   fin/sc(w) pop only after all pv(w) popped;
        #   pv(w+1) pops only after all sc(w) popped (acc slot handoff).
        pending = []
        live_pv = {w: 0 for w in range(NWV)}
        live_sc = {w: 0 for w in range(-1, NWV)}
        live_sc[-1] = 0
        state = {"gi": 0}
        lt_holder = {}

        def emit_lt(w, tb):
            s0 = w * 512
            lt = ltpool.tile([128, 1024], fp32, tag="lt",
                             name=f"lt_{w}_{tb}")
            for h in range(2):
                nc.tensor.matmul(
                    lt[:, h * 512:(h + 1) * 512],
                    kT_sb[DK * h:DK * (h + 1), tb * 128:(tb + 1) * 128],
                    qT_sb[DK * h:DK * (h + 1), s0:s0 + 512],
                    start=True, stop=True,
                    tile_position=(DK * h, 0),
                )
            return lt

        def pv_thunk(w, tb, pt, accs):
            def run():
                if w == 0 and V_TRACK:
                    ensure_v(tb)
                    ensure_v(tb + 4)
                if tb == 0:
                    accs.append(accpool.tile([128, 2, 4, 128], fp32,
                                             tag="acc", name=f"acc_{w}"))
                acc = accs[0]
                p_all = (pt[:].bitcast(bf16) if pt.tensor.dtype == i16
                         else pt[:])
                for h in range(2):
                    for k in range(4):
                        # one accumulation group per PSUM zero-region (the
                        # 2KB bank holding all 4 k-slices of head h): start
                        # marks the whole bank pending-zero, so only the
                        # first slice may start and only the last may stop;
                        # the other tb==0 writes zero-on-first-touch.
                        nc.tensor.matmul(
                            acc[:, h, k, 0:DK + 1],
                            p_all[:, h * 512 + k * 128:h * 512 + (k + 1) * 128],
                            v_aug[:, tb, h, :],
                            start=(tb == 0 and k == 0),
                            stop=(tb == NT - 1 and k == 3),
                            skip_group_check=(k != 0),
                        )
            return run

        def finalize_thunks(w, accs):
            rden = dpool.tile([128, 2, 4], fp32, tag="rden", name=f"rden_{w}")
            attn_all = attnpool.tile([128, 2, 4, DK], bf16, tag="attn",
                                     name=f"attn_{w}")
            atT = {}

            def recip():
                nc.vector.reciprocal(rden[:], accs[0][:, :, :, DK])

            def scale_ks(k0, k1):
                # one DVE op normalizes all (h, k0:k1) blocks: rden broadcast
                # along dk via a stride-0 free dim
                nc.vector.tensor_tensor(
                    attn_all[:, :, k0:k1, :], accs[0][:, :, k0:k1, 0:DK],
                    rden[:, :, k0:k1, None].to_broadcast([128, 2, k1 - k0, DK]),
                    ALU.mult)

            def scale_act(k):
                # last wave: late blocks on the otherwise-idle ACT so their
                # transposes unblock without waiting out DVE's queue
                for h in range(2):
                    nc.scalar.activation(attn_all[:, h, k, :],
                                         accs[0][:, h, k, 0:DK],
                                         AF.Copy, scale=rden[:, h, k:k + 1])

            last = w == NWV - 1

            def transp(h, k):
                if h == 0:
                    atT[k] = atTpool.tile([128, 128], bf16, tag="atT",
                                          name=f"atT_{w}_{k}")
                tps = pspool.tile([DK, 128], bf16, tag="ps",
                                  name=f"tps_{w}_{h}_{k}")
                nc.tensor.transpose(tps[:], attn_all[:, h, k, :], id_sb[:])
                nc.vector.tensor_copy(atT[k][DK * h:DK * (h + 1), :],
                                      tps[:])

            def yblock(k):
                # one combined [128,1024] output + ONE y DMA per s-block:
                # HWDGE serializes issues at ~625ns, so halving the DMA
                # count shortens the end-of-kernel trickle
                b = w * 4 + k
                yo = yopool.tile([128, 1024], bf16, tag="yo",
                                 name=f"yo_{b}")
                if last:
                    # the lt slots are dead after the final exp and the acc
                    # bank after the scales: each block gets a full 2-bank
                    # slot so both column-halves issue back-to-back, and
                    # the copies split across the idle ACT and DVE
                    pool, tag = ((ltpool, "lt"), (accpool, "acc"),
                                 (ltpool, "lt"), (ltpool, "lt"))[k]
                    yp = pool.tile([128, 1024], fp32, tag=tag,
                                   name=f"yp_{b}")
                    for jc in range(2):
                        nc.tensor.matmul(
                            yp[:, jc * 512:(jc + 1) * 512], atT[k][:],
                            wo_sb[:, jc * 512:(jc + 1) * 512],
                            start=True, stop=True)
                    nc.scalar.copy(yo[:, 0:512], yp[:, 0:512])
                    nc.vector.tensor_copy(yo[:, 512:1024], yp[:, 512:1024])
                else:
                    for jc in range(2):
                        yp = pspool.tile([128, 512], fp32, tag="ps",
                                         name=f"yp_{b}_{jc}")
                        nc.tensor.matmul(
                            yp[:], atT[k][:],
                            wo_sb[:, jc * 512:(jc + 1) * 512],
                            start=True, stop=True)
                        # split the two copies across ACT/DVE for balance
                        if jc == 0:
                            nc.scalar.copy(yo[:, 0:512], yp[:])
                        else:
                            nc.vector.tensor_copy(
                                yo[:, jc * 512:(jc + 1) * 512], yp[:])
                nc.sync.dma_start(y[b * 128:(b + 1) * 128, :], yo[:])

            # (floor_offset, pe_cost_ns, thunk): scales all run first (they
            # are acc's only readers, so the next wave's PV start unblocks
            # early); transposes and output blocks then trickle so the
            # finalize never oversubscribes an iteration's PE slack
            thunks = [(1, 10, "sc", recip)]
            if last:
                thunks.append((2, 10, "sc", lambda: scale_ks(0, 2)))
                thunks.append((3, 10, "sc", lambda: scale_act(2)))
                thunks.append((4, 10, "sc", lambda: scale_act(3)))
            else:
                thunks.append((2, 10, "sc", lambda: scale_ks(0, 4)))
            for k in range(4):
                thunks.append((5 + 6 * k, 110, "fin",
                               lambda k=k: (transp(0, k), transp(1, k))))
                thunks.append((7 + 6 * k, 430, "fin",
                               lambda k=k: yblock(k)))
            return thunks

        def emit_iter(w, tb, accs):
            gi = state["gi"]
            lag = {0: LAG_W0, 1: LAG_W1, NWV - 1: LAG_LAST}.get(w, LAG)
            if w >= 2 and w != NWV - 1:
                # smooth the wave-boundary acc handoff (PVlast(w-1) -> recip
                # -> scale -> PV0(w), accpool bufs=1): give the first PVs of
                # the wave extra lag so the serial chain hides under lt/proj
                # work instead of stalling the PE
                lag = max(LAG, 6 - tb)
            lt = lt_holder.pop("lt")
            dve_tbs = DVE_TBS_W0 if w == 0 else DVE_TBS
            # exp in per-head halves: the lt slot's h0 half frees one
            # exp-half earlier, cutting the lt->exp->slot-free round trip
            # below the PE iteration time (the slot ping-pong is only 2 deep;
            # PSUM can't fit 3) — subtile deps let lt(i+2)'s h0 matmul start
            # as soon as exp(i)'s h0 half completes
            if tb in dve_tbs:
                # Schraudolph exp on DVE: affine into int16, bitcast bf16
                pt = ptpool.tile([128, 1024], i16, tag="pt",
                                 name=f"pt_{w}_{tb}")
                for hh in range(2):
                    nc.vector.tensor_scalar(
                        pt[:, hh * 512:(hh + 1) * 512],
                        lt[:, hh * 512:(hh + 1) * 512], SCH_A, SCH_B,
                        ALU.mult, ALU.add)
            else:
                pt = ptpool.tile([128, 1024], bf16, tag="pt",
                                 name=f"pt_{w}_{tb}")
                for hh in range(2):
                    nc.scalar.activation(pt[:, hh * 512:(hh + 1) * 512],
                                         lt[:, hh * 512:(hh + 1) * 512],
                                         AF.Exp, scale=0.125)
            if tb + 1 < NT:
                lt_holder["lt"] = emit_lt(w, tb + 1)
            elif w + 1 < NWV:
                lt_holder["lt"] = emit_lt(w + 1, 0)
            live_pv[w] += 1
            pending.append([gi + lag, 644 if (w == 0 and V_TRACK) else 217,
                            "pv", w, pv_thunk(w, tb, pt, accs)])
            # Pop READY items anywhere in the list (a far-future finalize
            # floor must not head-block the PV stream), but cap the popped
            # PE-ns per iteration: the exp cadence only tolerates ~1.2us of
            # PE work between consecutive lt emissions before ACT stalls.
            budget = PE_BUDGET
            i = 0
            while i < len(pending):
                floor, cost, kind, wv, t = pending[i]
                ok = floor <= gi and (cost <= budget or floor <= gi - 12)
                if ok and kind == "pv":
                    ok = live_sc[wv - 1] == 0
                elif ok:
                    ok = live_pv[wv] == 0
                if ok:
                    pending.pop(i)
                    t()
                    budget -= cost
                    if kind == "pv":
                        live_pv[wv] -= 1
                    elif kind == "sc":
                        live_sc[wv] -= 1
                else:
                    i += 1
            # projection deadlines are HARD (logits read qT/kT at fixed
            # iterations): pop regardless of remaining budget
            pops = 0
            while proj_q and proj_q[0][0] <= gi and pops < 2:
                proj_q.popleft()[2]()
                pops += 1
            state["gi"] = gi + 1

        # ---- main loop ----
        lt_holder["lt"] = emit_lt(0, 0)
        for w in range(NWV):
            accs = []
            for tb in range(NT):
                emit_iter(w, tb, accs)
            for off, cost, kind, t in finalize_thunks(w, accs):
                if kind == "sc":
                    live_sc[w] += 1
                pending.append([state["gi"] + LAG - 2 + off, cost, kind, w, t])
        while proj_q:
            proj_q.popleft()[2]()
        # final drain: keep list order within a wave; gates are satisfied
        # by construction (pv entries precede sc precede fin per wave)
        for e in pending:
            e[4]()

    _split_multi_waits(nc, mybir)
    nc.finalize()
    return nc


def _get_nc():
    if "nc" not in _NC_CACHE:
        _NC_CACHE["nc"] = _build_nc()
    return _NC_CACHE["nc"]


def _relay(wT):
    """[1024 d, 128 m] -> [p, c*m] with wT[c*128+p, m] at [p, c, m]: every
    DMA descriptor becomes a contiguous 2KB run."""
    return np.ascontiguousarray(
        wT.reshape(8, 128, HD).transpose(1, 0, 2).reshape(128, 8 * HD))


def _in_maps(x, Wq, Wk, Wv, Wo):
    import ml_dtypes
    bf16 = ml_dtypes.bfloat16
    xT = np.ascontiguousarray(np.asarray(x, np.float32).T).astype(bf16)
    ident = np.eye(128, dtype=np.float32).astype(bf16)
    maps = []
    for c in range(NCORES):
        sl = slice(HD * c, HD * (c + 1))
        maps.append(dict(
            xT=xT,
            wqT=_relay(np.asarray(Wq)[sl, :].T.astype(bf16)),
            wkT=_relay(np.asarray(Wk)[sl, :].T.astype(bf16)),
            wvT=_relay(np.asarray(Wv)[sl, :].T.astype(bf16)),
            woT=np.ascontiguousarray(np.asarray(Wo)[:, sl].T).astype(bf16),
            ident=ident,
        ))
    return maps


def kernel(x, Wq, Wk, Wv, Wo):
    from concourse.bass_utils import run_bass_kernel_spmd

    x = np.asarray(x, dtype=np.float32)
    nc = _get_nc()
    res = run_bass_kernel_spmd(nc, _in_maps(x, Wq, Wk, Wv, Wo),
                               list(range(NCORES)))
    out = np.zeros((S, D), np.float32)
    for rr in res.results:
        out += np.asarray(rr["y"], dtype=np.float32)
    return out



# revision 18
# speedup vs baseline: 1.2654x; 1.2654x over previous
"""Multi-head attention (S=4096, D=1024, H=16) on 8 trn2 NeuronCores.

Sharding: 2 heads per core (tensor-parallel on Q/K/V column splits and
dense row split). Each core computes a partial [S, D] output; host sums
the 8 partials (the unshard step for row-parallel TP).

Per-core design (bf16 operands, fp32 PSUM accumulate):
  The ACT engine's exp over the S*S*2 logits (256 x [128,1024] tiles,
  ~1.04us each) is the hard floor (~266us busy); everything else is
  scheduled to hide inside it (this build: ~292us total = 10.8us
  startup + 266us exp + ~2.5us exp stalls + ~12.4us finalize tail).
  - logits: lt[128t, 2h*512s] = k-block^T q-window per head (2 matmuls,
    512 rows each).
  - PV streams v, not P: stationary = pt s-block [128t,128s], moving =
    v_aug[128t,65] -> acc[128s,65] accumulated over 32 t-blocks (65
    rows/matmul instead of 512); column 64 of v_aug is ones so the
    softmax denominator lands in acc[...,64] already laid out per
    s-partition. One accumulation group per PSUM 2KB zero-region: only
    (tb0,k0) starts and (tb31,k3) stops each head's bank.
  - normalization: reciprocal of the denominator, then per-partition
    tensor_scalar_mul fused into the acc->SBUF copy.
  - v-projection emits v directly in [s-part, dk] layout (stationary =
    x-block, moving = Wv chunk): no PE transposes of V.
  - output projection: per s-block, transpose normalized attn
    ([128s,64]->[64dk,128s] via PE, 1 cyc/row in bf16), then
    yp[128s,512d] = attnT^T wo; y is bf16, host sums partials in fp32.
  PSUM (8 banks): lt 2x2 (ping-pong), acc 2 (padded [128,2,4,128],
  single-buffered across waves), 2-slot scratch ring shared by proj
  psum / transposes / yp; the last wave's yp also rotates through the
  then-dead lt+acc banks so the tail pipelines 3-deep.
  Schedule: one exp per iteration paces everything. Between consecutive
  lt emissions the PE tolerates only ~1.2us of other work, so PV /
  finalize thunks pop from a floor-gated list under a PE-ns budget;
  projection deadlines are hard (logits read qT/kT at fixed iterations)
  and ordering gates enforce pv(w) < scales(w) < pv(w+1) (acc handoff).
  GPSIMD cannot touch PSUM, so all PSUM evacuation is DVE/ACT.
  Startup: matmul cost is priced at decode time ~36 instructions ahead
  of execution, so 8 big + 36 tiny memset-fed warm-up matmuls both ramp
  the PE p-state and flush the decode queue before the first real
  projections; DMA order puts wk/wq/x-piece-0 first (HWDGE serializes
  at ~625ns per dma_start) and weights are host-re-laid so every DMA
  descriptor is a contiguous 2KB run.
"""

import numpy as np
from collections import deque
from contextlib import ExitStack

S = 4096
D = 1024
NCORES = 8
HD = 128  # head-dim span per core (2 heads x 64)
DK = 64
NT = S // 128   # 32 t-blocks
NWV = 8         # s-waves of 512
LAG_W0 = 24   # wave-0 PV lag: pushes PV+v work past the k/q-proj burst
LAG = 2       # steady-state PV lag (lt(i+1) is emitted before the
              # pops, so a briefly-waiting PV never blocks logits)
LAG_LAST = 2  # final wave drains tight to shorten the tail
PE_BUDGET = 600   # max popped PE-ns per iteration (guards the exp cadence)
V_TRACK = True    # v-blocks emitted from inside wave-0 PV pops (+4 lookahead)
Q_MID = False     # q pieces at wave end (mid-wave collides with finalize)
LAG_W1 = 14       # wave-1 PV lag (wave-0 backlog still draining)

# Schraudolph exp-on-DVE: i16 = trunc/round(A*(0.125*lt) + B); bitcast bf16
# gives e^z * (1 + O(3%)).  A = 128*log2(e) folded with the 0.125 logit
# scale; B centers the linear-in-mantissa interpolation error (calibrated
# against the real convert rounding via test.py sweep).
SCH_A = float(128.0 / np.log(2.0) * 0.125)
SCH_B = 16256.0 - 5.0
# t-blocks whose exp runs on DVE (Schraudolph) instead of ACT, per wave.
# 12/32 offload in an A,A,D-ish weave: ACT does 20 exps/wave (~21us), DVE
# ~14us + evacuations; both below the PE wave time (~27.8us), making PE the
# pacing engine.  tb 0-3 stay on ACT so the wave-boundary recip/scale (the
# serial acc handoff, accpool bufs=1) isn't queued behind a 1.2us DVE exp.
DVE_TBS = {4, 6, 9, 11, 14, 16, 19, 21, 24, 26, 29, 31}
DVE_TBS_W0 = {14, 17, 20, 23, 26, 29}  # wave 0: DVE busy with v/proj backlog

_NC_CACHE = {}


def _split_multi_waits(nc, mybir):
    """This walrus build encodes at most ~2 sync commands per instruction
    (1 for matmul/drain). Keep <=1 wait on every compute/DMA instruction and
    move the rest into standalone dual-condition EventSemaphore instructions
    inserted immediately before it on the same engine (same wait point, so
    semantics are unchanged)."""
    n = 0
    used = set()
    for b in nc.m.functions[0].blocks:
        for inst in b.instructions:
            si = inst.sync_info
            if si:
                for w in (si.on_wait or []):
                    used.add(w.id)
                for u in (si.on_update or []):
                    used.add(u.id)
    free_ids = [i for i in range(max(used) + 1, max(used) + 32)]
    sems = {}

    def eng_sem(eng):
        if eng not in sems:
            sems[eng] = (free_ids.pop(0), f"wsplit_{len(sems)}")
        return sems[eng]

    for b in nc.m.functions[0].blocks:
        il = b.instructions
        new = []
        for inst in il:
            si = inst.sync_info
            waits = list(si.on_wait) if si and si.on_wait else []
            upds = list(si.on_update) if si and si.on_update else []
            if type(inst).__name__ == "InstEventSemaphore":
                new.append(inst)
                continue
            if len(waits) > 1:
                excess, keep = waits[:-1], waits[-1:]
                for i in range(0, len(excess), 2):
                    sid, sname = eng_sem(inst.engine)
                    ev = mybir.InstEventSemaphore(
                        name=f"{inst.name}_ws{i}", engine=inst.engine,
                        ins=[], outs=[],
                        sync_info=mybir.SyncInfo(
                            on_wait=excess[i:i + 2],
                            on_update=[mybir.SyncUpdate(
                                sync_type="semaphore", id=sid,
                                ant_name=sname, update_mode="sem-inc",
                                update_value=1, update_reg=None)]))
                    new.append(ev)
                    n += 1
                inst.sync_info = mybir.SyncInfo(on_wait=keep, on_update=upds)
            new.append(inst)
        il[:] = new
    return n


def _build_nc():
    import concourse.bass as bass
    import concourse.tile as tile
    import concourse.mybir as mybir

    fp32 = mybir.dt.float32
    bf16 = mybir.dt.bfloat16
    i16 = mybir.dt.int16
    AF = mybir.ActivationFunctionType
    ALU = mybir.AluOpType

    nc = bass.Bass()
    xT = nc.dram_tensor("xT", [D, S], bf16, kind="ExternalInput")
    # weights pre-laid host-side as [p, c, m] so each DMA descriptor is a
    # contiguous 2KB run (256B descriptors pay a 2x latency penalty)
    wqT = nc.dram_tensor("wqT", [128, 8 * HD], bf16, kind="ExternalInput")
    wkT = nc.dram_tensor("wkT", [128, 8 * HD], bf16, kind="ExternalInput")
    wvT = nc.dram_tensor("wvT", [128, 8 * HD], bf16, kind="ExternalInput")
    woT = nc.dram_tensor("woT", [HD, D], bf16, kind="ExternalInput")
    ident = nc.dram_tensor("ident", [128, 128], bf16, kind="ExternalInput")
    y = nc.dram_tensor("y", [S, D], bf16, kind="ExternalOutput")

    with tile.TileContext(nc) as tc, ExitStack() as ctx, \
         nc.allow_low_precision(reason="bf16 operands within rel-err budget"):
        sb = ctx.enter_context(tc.tile_pool(name="sb", bufs=1))
        qT_sb = sb.tile([128, S], bf16, tag="qT")
        kT_sb = sb.tile([128, S], bf16, tag="kT")
        # v_aug[:, tb, h, 0:64] = v block for head h; [..., 64] = ones
        v_aug = sb.tile([128, NT, 2, DK + 1], bf16, tag="vaug")
        wq_sb = sb.tile([128, 8, HD], bf16, tag="wq")
        wk_sb = sb.tile([128, 8, HD], bf16, tag="wk")
        wv_sb = sb.tile([128, 8, HD], bf16, tag="wv")
        wo_sb = sb.tile([HD, D], bf16, tag="wo")
        id_sb = sb.tile([128, 128], bf16, tag="id")
        warm = sb.tile([1, DK], fp32, tag="warm")
        wsrc = sb.tile([128, 512], bf16, tag="wsrc")

        xpool = ctx.enter_context(tc.tile_pool(name="xpool", bufs=8))
        ptpool = ctx.enter_context(tc.tile_pool(name="ptpool", bufs=LAG_W0 + 14))
        attnpool = ctx.enter_context(tc.tile_pool(name="attnpool", bufs=2))
        atTpool = ctx.enter_context(tc.tile_pool(name="atTpool", bufs=4))
        yopool = ctx.enter_context(tc.tile_pool(name="yopool", bufs=8))
        dpool = ctx.enter_context(tc.tile_pool(name="dpool", bufs=2))
        ltpool = ctx.enter_context(
            tc.tile_pool(name="ltpool", bufs=4, space="PSUM"))
        accpool = ctx.enter_context(
            tc.tile_pool(name="accpool", bufs=1, space="PSUM"))
        pspool = ctx.enter_context(
            tc.tile_pool(name="pspool", bufs=2, space="PSUM"))

        # ---- startup DMAs: critical path (wk, wq, x piece 0) first ----
        nc.sync.dma_start(wk_sb[:], wkT.rearrange("p (c m) -> p c m", c=8))
        nc.sync.dma_start(wq_sb[:], wqT.rearrange("p (c m) -> p c m", c=8))
        xTr = xT.rearrange("(c p) s -> p c s", p=128)
        xqs = {}

        def load_xq(p):
            xq = xpool.tile([128, 8, 512], bf16, tag="xq", name=f"xq_{p}")
            for c in range(0, 8, 2):
                nc.sync.dma_start(xq[:, c:c + 2, :],
                                  xTr[:, c:c + 2, p * 512:(p + 1) * 512])
            xqs[p] = xq

        load_xq(0)
        nc.sync.dma_start(wv_sb[:], wvT.rearrange("p (c m) -> p c m", c=8))
        nc.sync.dma_start(wo_sb[:], woT[:])
        nc.sync.dma_start(id_sb[:], ident[:])
        nc.vector.memset(v_aug[:, :, :, DK], 1.0)
        # Warm-up fed by an on-chip memset (no DMA dependency). Matmul cost
        # is priced at DECODE time, which runs ~36 instructions (exec-queue
        # depth) ahead of execution: to get the real projection matmuls
        # priced at the full-speed p-state, the PE must (a) be continuously
        # busy >3us and (b) have >=36 instructions in flight before them.
        # 8 big warms ramp the clock, 36 tiny ones flush the decode queue.
        nc.vector.memset(wsrc[:], 0.5)
        nc.scalar.activation(warm[:], wsrc[0:1, 0:DK], AF.Exp, scale=0.125)
        warm_slots = [
            pspool.tile([128, 512], fp32, tag="ps", name=f"warmps_{i}")
            for i in range(2)]
        warm_lt = [
            ltpool.tile([128, 512], fp32, tag="lt", name=f"warmlt_{i}")
            for i in range(4)]
        for i in range(8):
            wp = (warm_slots[i % 2][:, 0:512] if i % 4 < 2
                  else warm_lt[i % 4][:])
            nc.tensor.matmul(wp, wsrc[:, 0:128], wsrc[:, 0:512],
                             start=True, stop=True)
        for i in range(36):
            wp = (warm_slots[i % 2][:, 0:32] if i % 4 < 2
                  else warm_lt[i % 4][:, 0:32])
            nc.tensor.matmul(wp, wsrc[:, 0:128], wsrc[:, 0:32],
                             start=True, stop=True)
        # remaining x pieces; all 8 stay resident (q-proj of piece p runs
        # ~4 waves after its k-proj, so slots can't rotate)
        for p in range(1, 8):
            load_xq(p)

        # ---- projection pieces (as resumable quarter thunks) ----
        def qk_proj_thunks(p, w_sb, dst):
            """Four thunks of 2 chunks each for dst[:, p*512:(p+1)*512]."""
            hold = {}

            def quarter(i):
                def run():
                    if i == 0:
                        hold["ps"] = pspool.tile(
                            [128, 512], fp32, tag="ps",
                            name=f"pqk_{dst.tensor.name}_{p}")
                    ps = hold["ps"]
                    for c in range(2 * i, 2 * i + 2):
                        nc.tensor.matmul(ps[:], w_sb[:, c, :],
                                         xqs[p][:, c, :], start=(c == 0),
                                         stop=(c == 7))
                    if i == 3:
                        nc.vector.tensor_copy(
                            dst[:, p * 512:(p + 1) * 512], hold.pop("ps")[:])
                return run
            return [quarter(i) for i in range(4)]

        def v_block_thunk(tb):
            def run():
                p, i = tb // 4, tb % 4
                vps = pspool.tile([128, 128], fp32, tag="ps",
                                  name=f"vps_{tb}")
                for c in range(8):
                    nc.tensor.matmul(
                        vps[:], xqs[p][:, c, i * 128:(i + 1) * 128],
                        wv_sb[:, c, :], start=(c == 0), stop=(c == 7))
                nc.vector.tensor_copy(
                    v_aug[:, tb, :, 0:DK],
                    vps[:].rearrange("p (h d) -> p h d", h=2))
            return run

        # ---- startup projections: piece 0 of q and k, interleaved per
        # x-chunk so matmuls start as soon as each chunk DMA lands; the
        # two PSUM->SBUF copies run on different engines in parallel ----
        pq0 = pspool.tile([128, 512], fp32, tag="ps", name="pq0")
        pk0 = pspool.tile([128, 512], fp32, tag="ps", name="pk0")
        xq0 = xqs[0]
        for c in range(8):
            nc.tensor.matmul(pq0[:], wq_sb[:, c, :], xq0[:, c, :],
                             start=(c == 0), stop=(c == 7))
            nc.tensor.matmul(pk0[:], wk_sb[:, c, :], xq0[:, c, :],
                             start=(c == 0), stop=(c == 7))
        nc.scalar.copy(qT_sb[:, 0:512], pq0[:])
        # lt(0,0) needs only k block 0: give it its own small copy
        nc.vector.tensor_copy(kT_sb[:, 0:128], pk0[:, 0:128])
        nc.vector.tensor_copy(kT_sb[:, 128:512], pk0[:, 128:512])

        # deadline-ordered weave of the remaining projection work; each
        # entry is [deadline, pe_cost_ns, thunk]
        proj_q = deque()
        for p in range(1, 8):
            for j, t in enumerate(qk_proj_thunks(p, wk_sb, kT_sb)):
                proj_q.append([4 * p - 5 + j, 427, t])
        # v blocks 0..3 by deadline; the rest are emitted from inside the
        # wave-0 PV pops with a +4 lookahead, auto-tracking the drain rate
        v_done = set()

        def ensure_v(tb):
            if tb < NT and tb not in v_done:
                v_done.add(tb)
                v_block_thunk(tb)()

        n_pre_v = 4 if V_TRACK else NT
        for tb in range(n_pre_v):
            proj_q.append([LAG_W0 - 6 + tb, 427, lambda tb=tb: ensure_v(tb)])
        # q piece w must be fully copied before lt(w, 0) is emitted at
        # gi 32w-1: the last quarter lands at base+6 <= 32w-3
        for j, t in enumerate(qk_proj_thunks(1, wq_sb, qT_sb)):
            proj_q.append([(16 if Q_MID else 22) + 2 * j, 427, t])
        for w in range(2, NWV):
            for j, t in enumerate(qk_proj_thunks(w, wq_sb, qT_sb)):
                base = 32 * (w - 1) + (10 if Q_MID else 22)
                proj_q.append([base + 2 * j, 427, t])
        proj_q = deque(sorted(proj_q, key=lambda e: e[0]))

        # ---- attention machinery ----
        # pending: [floor, cost, kind, wave, thunk]; kind "pv" | "sc" | "fin".
        # Ordering gates (enforced at pop time, not just via floors):
        #   fin/sc(w) pop only after all pv(w) popped;
        #   pv(w+1) pops only after all sc(w) popped (acc slot handoff).
        pending = []
        live_pv = {w: 0 for w in range(NWV)}
        live_sc = {w: 0 for w in range(-1, NWV)}
        live_sc[-1] = 0
        state = {"gi": 0}
        lt_holder = {}

        def emit_lt(w, tb):
            # per-head lt tiles (1 PSUM bank each): each head's
            # lt -> exp -> slot-free chain ping-pongs independently across
            # 2 of the 4 slots, halving the latency the cadence must absorb
            s0 = w * 512
            lts = []
            for h in range(2):
                lt_h = ltpool.tile([128, 512], fp32, tag="lt",
                                   name=f"lt_{w}_{tb}_{h}")
                nc.tensor.matmul(
                    lt_h[:],
                    kT_sb[DK * h:DK * (h + 1), tb * 128:(tb + 1) * 128],
                    qT_sb[DK * h:DK * (h + 1), s0:s0 + 512],
                    start=True, stop=True,
                    tile_position=(DK * h, 0),
                )
                lts.append(lt_h)
            return lts

        def pv_thunk(w, tb, pt, accs):
            def run():
                if w == 0 and V_TRACK:
                    ensure_v(tb)
                    ensure_v(tb + 4)
                if tb == 0:
                    accs.append(accpool.tile([128, 2, 4, 128], fp32,
                                             tag="acc", name=f"acc_{w}"))
                acc = accs[0]
                p_all = (pt[:].bitcast(bf16) if pt.tensor.dtype == i16
                         else pt[:])
                for h in range(2):
                    for k in range(4):
                        # one accumulation group per PSUM zero-region (the
                        # 2KB bank holding all 4 k-slices of head h): start
                        # marks the whole bank pending-zero, so only the
                        # first slice may start and only the last may stop;
                        # the other tb==0 writes zero-on-first-touch.
                        nc.tensor.matmul(
                            acc[:, h, k, 0:DK + 1],
                            p_all[:, h * 512 + k * 128:h * 512 + (k + 1) * 128],
                            v_aug[:, tb, h, :],
                            start=(tb == 0 and k == 0),
                            stop=(tb == NT - 1 and k == 3),
                            skip_group_check=(k != 0),
                        )
            return run

        def finalize_thunks(w, accs):
            rden = dpool.tile([128, 2, 4], fp32, tag="rden", name=f"rden_{w}")
            attn_all = attnpool.tile([128, 2, 4, DK], bf16, tag="attn",
                                     name=f"attn_{w}")
            atT = {}

            def recip():
                nc.vector.reciprocal(rden[:], accs[0][:, :, :, DK])

            def scale_ks(k0, k1):
                # one DVE op normalizes all (h, k0:k1) blocks: rden broadcast
                # along dk via a stride-0 free dim
                nc.vector.tensor_tensor(
                    attn_all[:, :, k0:k1, :], accs[0][:, :, k0:k1, 0:DK],
                    rden[:, :, k0:k1, None].to_broadcast([128, 2, k1 - k0, DK]),
                    ALU.mult)

            def scale_act(k):
                # last wave: late blocks on the otherwise-idle ACT so their
                # transposes unblock without waiting out DVE's queue
                for h in range(2):
                    nc.scalar.activation(attn_all[:, h, k, :],
                                         accs[0][:, h, k, 0:DK],
                                         AF.Copy, scale=rden[:, h, k:k + 1])

            last = w == NWV - 1

            def transp(h, k):
                if h == 0:
                    atT[k] = atTpool.tile([128, 128], bf16, tag="atT",
                                          name=f"atT_{w}_{k}")
                tps = pspool.tile([DK, 128], bf16, tag="ps",
                                  name=f"tps_{w}_{h}_{k}")
                nc.tensor.transpose(tps[:], attn_all[:, h, k, :], id_sb[:])
                nc.vector.tensor_copy(atT[k][DK * h:DK * (h + 1), :],
                                      tps[:])

            def yblock(k):
                # one combined [128,1024] output + ONE y DMA per s-block:
                # HWDGE serializes issues at ~625ns, so halving the DMA
                # count shortens the end-of-kernel trickle
                b = w * 4 + k
                yo = yopool.tile([128, 1024], bf16, tag="yo",
                                 name=f"yo_{b}")
                if last:
                    # the lt slots are dead after the final exp and the acc
                    # banks after the scales: draw per-half yp banks from
                    # them so the tail pipelines 4+ deep, and split the
                    # copies across the idle ACT and DVE
                    if k == 1:
                        ypt = accpool.tile([128, 2, 512], fp32, tag="acc",
                                           name=f"yp_{b}")
                        yps = [ypt[:, jc, :] for jc in range(2)]
                    else:
                        yps = [ltpool.tile([128, 512], fp32, tag="lt",
                                           name=f"yp_{b}_{jc}")[:]
                               for jc in range(2)]
                    for jc in range(2):
                        nc.tensor.matmul(
                            yps[jc], atT[k][:],
                            wo_sb[:, jc * 512:(jc + 1) * 512],
                            start=True, stop=True)
                    nc.scalar.copy(yo[:, 0:512], yps[0])
                    nc.vector.tensor_copy(yo[:, 512:1024], yps[1])
                else:
                    for jc in range(2):
                        yp = pspool.tile([128, 512], fp32, tag="ps",
                                         name=f"yp_{b}_{jc}")
                        nc.tensor.matmul(
                            yp[:], atT[k][:],
                            wo_sb[:, jc * 512:(jc + 1) * 512],
                            start=True, stop=True)
                        # split the two copies across ACT/DVE for balance
                        if jc == 0:
                            nc.scalar.copy(yo[:, 0:512], yp[:])
                        else:
                            nc.vector.tensor_copy(
                                yo[:, jc * 512:(jc + 1) * 512], yp[:])
                nc.sync.dma_start(y[b * 128:(b + 1) * 128, :], yo[:])

            # (floor_offset, pe_cost_ns, thunk): scales all run first (they
            # are acc's only readers, so the next wave's PV start unblocks
            # early); transposes and output blocks then trickle so the
            # finalize never oversubscribes an iteration's PE slack
            thunks = [(1, 10, "sc", recip)]
            if last:
                thunks.append((2, 10, "sc", lambda: scale_ks(0, 2)))
                thunks.append((3, 10, "sc", lambda: scale_act(2)))
                thunks.append((4, 10, "sc", lambda: scale_act(3)))
            else:
                thunks.append((2, 10, "sc", lambda: scale_ks(0, 4)))
            for k in range(4):
                thunks.append((5 + 6 * k, 110, "fin",
                               lambda k=k: (transp(0, k), transp(1, k))))
                thunks.append((7 + 6 * k, 430, "fin",
                               lambda k=k: yblock(k)))
            return thunks

        def emit_iter(w, tb, accs):
            gi = state["gi"]
            lag = {0: LAG_W0, 1: LAG_W1, NWV - 1: LAG_LAST}.get(w, LAG)
            if w >= 2 and w != NWV - 1:
                # smooth the wave-boundary acc handoff (PVlast(w-1) -> recip
                # -> scale -> PV0(w), accpool bufs=1): give the first PVs of
                # the wave extra lag so the serial chain hides under lt/proj
                # work instead of stalling the PE
                lag = max(LAG, 6 - tb)
            lt = lt_holder.pop("lt")
            dve_tbs = DVE_TBS_W0 if w == 0 else DVE_TBS
            # exp in per-head halves: the lt slot's h0 half frees one
            # exp-half earlier, cutting the lt->exp->slot-free round trip
            # below the PE iteration time (the slot ping-pong is only 2 deep;
            # PSUM can't fit 3) — subtile deps let lt(i+2)'s h0 matmul start
            # as soon as exp(i)'s h0 half completes
            if tb in dve_tbs:
                # Schraudolph exp on DVE: affine into int16, bitcast bf16
                pt = ptpool.tile([128, 1024], i16, tag="pt",
                                 name=f"pt_{w}_{tb}")
                for hh in range(2):
                    nc.vector.tensor_scalar(
                        pt[:, hh * 512:(hh + 1) * 512],
                        lt[hh][:], SCH_A, SCH_B,
                        ALU.mult, ALU.add)
            else:
                pt = ptpool.tile([128, 1024], bf16, tag="pt",
                                 name=f"pt_{w}_{tb}")
                for hh in range(2):
                    nc.scalar.activation(pt[:, hh * 512:(hh + 1) * 512],
                                         lt[hh][:],
                                         AF.Exp, scale=0.125)
            if tb + 1 < NT:
                lt_holder["lt"] = emit_lt(w, tb + 1)
            elif w + 1 < NWV:
                lt_holder["lt"] = emit_lt(w + 1, 0)
            live_pv[w] += 1
            pending.append([gi + lag, 644 if (w == 0 and V_TRACK) else 217,
                            "pv", w, pv_thunk(w, tb, pt, accs)])
            # Pop READY items anywhere in the list (a far-future finalize
            # floor must not head-block the PV stream), but cap the popped
            # PE-ns per iteration: the exp cadence only tolerates ~1.2us of
            # PE work between consecutive lt emissions before ACT stalls.
            budget = PE_BUDGET
            i = 0
            while i < len(pending):
                floor, cost, kind, wv, t = pending[i]
                ok = floor <= gi and (cost <= budget or floor <= gi - 12)
                if ok and kind == "pv":
                    ok = live_sc[wv - 1] == 0
                elif ok:
                    ok = live_pv[wv] == 0
                if ok:
                    pending.pop(i)
                    t()
                    budget -= cost
                    if kind == "pv":
                        live_pv[wv] -= 1
                    elif kind == "sc":
                        live_sc[wv] -= 1
                else:
                    i += 1
            # projection deadlines are HARD (logits read qT/kT at fixed
            # iterations): pop regardless of remaining budget
            pops = 0
            while proj_q and proj_q[0][0] <= gi and pops < 2:
                proj_q.popleft()[2]()
                pops += 1
            state["gi"] = gi + 1

        # ---- main loop ----
        lt_holder["lt"] = emit_lt(0, 0)
        for w in range(NWV):
            accs = []
            for tb in range(NT):
                emit_iter(w, tb, accs)
            for off, cost, kind, t in finalize_thunks(w, accs):
                if kind == "sc":
                    live_sc[w] += 1
                pending.append([state["gi"] + LAG - 2 + off, cost, kind, w, t])
        while proj_q:
            proj_q.popleft()[2]()
        # final drain: keep list order within a wave; gates are satisfied
        # by construction (pv entries precede sc precede fin per wave)
        for e in pending:
            e[4]()

    _split_multi_waits(nc, mybir)
    nc.finalize()
    return nc


def _get_nc():
    if "nc" not in _NC_CACHE:
        _NC_CACHE["nc"] = _build_nc()
    return _NC_CACHE["nc"]


def _relay(wT):
    """[1024 d, 128 m] -> [p, c*m] with wT[c*128+p, m] at [p, c, m]: every
    DMA descriptor becomes a contiguous 2KB run."""
    return np.ascontiguousarray(
        wT.reshape(8, 128, HD).transpose(1, 0, 2).reshape(128, 8 * HD))


def _in_maps(x, Wq, Wk, Wv, Wo):
    import ml_dtypes
    bf16 = ml_dtypes.bfloat16
    xT = np.ascontiguousarray(np.asarray(x, np.float32).T).astype(bf16)
    ident = np.eye(128, dtype=np.float32).astype(bf16)
    maps = []
    for c in range(NCORES):
        sl = slice(HD * c, HD * (c + 1))
        maps.append(dict(
            xT=xT,
            wqT=_relay(np.asarray(Wq)[sl, :].T.astype(bf16)),
            wkT=_relay(np.asarray(Wk)[sl, :].T.astype(bf16)),
            wvT=_relay(np.asarray(Wv)[sl, :].T.astype(bf16)),
            woT=np.ascontiguousarray(np.asarray(Wo)[:, sl].T).astype(bf16),
            ident=ident,
        ))
    return maps


def kernel(x, Wq, Wk, Wv, Wo):
    from concourse.bass_utils import run_bass_kernel_spmd

    x = np.asarray(x, dtype=np.float32)
    nc = _get_nc()
    res = run_bass_kernel_spmd(nc, _in_maps(x, Wq, Wk, Wv, Wo),
                               list(range(NCORES)))
    out = np.zeros((S, D), np.float32)
    for rr in res.results:
        out += np.asarray(rr["y"], dtype=np.float32)
    return out



# revision 24
# speedup vs baseline: 1.2680x; 1.0020x over previous
"""Multi-head attention (S=4096, D=1024, H=16) on 8 trn2 NeuronCores.

Sharding: 2 heads per core (tensor-parallel on Q/K/V column splits and
dense row split). Each core computes a partial [S, D] output; host sums
the 8 partials (the unshard step for row-parallel TP).

Per-core design (bf16 operands, fp32 PSUM accumulate):
  The ACT engine's exp over the S*S*2 logits (256 x [128,1024] tiles,
  ~1.04us each) is the hard floor (~266us busy); everything else is
  scheduled to hide inside it (this build: ~292us total = 10.8us
  startup + 266us exp + ~2.5us exp stalls + ~12.4us finalize tail).
  - logits: lt[128t, 2h*512s] = k-block^T q-window per head (2 matmuls,
    512 rows each).
  - PV streams v, not P: stationary = pt s-block [128t,128s], moving =
    v_aug[128t,65] -> acc[128s,65] accumulated over 32 t-blocks (65
    rows/matmul instead of 512); column 64 of v_aug is ones so the
    softmax denominator lands in acc[...,64] already laid out per
    s-partition. One accumulation group per PSUM 2KB zero-region: only
    (tb0,k0) starts and (tb31,k3) stops each head's bank.
  - normalization: reciprocal of the denominator, then per-partition
    tensor_scalar_mul fused into the acc->SBUF copy.
  - v-projection emits v directly in [s-part, dk] layout (stationary =
    x-block, moving = Wv chunk): no PE transposes of V.
  - output projection: per s-block, transpose normalized attn
    ([128s,64]->[64dk,128s] via PE, 1 cyc/row in bf16), then
    yp[128s,512d] = attnT^T wo; y is bf16, host sums partials in fp32.
  PSUM (8 banks): lt 2x2 (ping-pong), acc 2 (padded [128,2,4,128],
  single-buffered across waves), 2-slot scratch ring shared by proj
  psum / transposes / yp; the last wave's yp also rotates through the
  then-dead lt+acc banks so the tail pipelines 3-deep.
  Schedule: one exp per iteration paces everything. Between consecutive
  lt emissions the PE tolerates only ~1.2us of other work, so PV /
  finalize thunks pop from a floor-gated list under a PE-ns budget;
  projection deadlines are hard (logits read qT/kT at fixed iterations)
  and ordering gates enforce pv(w) < scales(w) < pv(w+1) (acc handoff).
  GPSIMD cannot touch PSUM, so all PSUM evacuation is DVE/ACT.
  Startup: matmul cost is priced at decode time ~36 instructions ahead
  of execution, so 8 big + 36 tiny memset-fed warm-up matmuls both ramp
  the PE p-state and flush the decode queue before the first real
  projections; DMA order puts wk/wq/x-piece-0 first (HWDGE serializes
  at ~625ns per dma_start) and weights are host-re-laid so every DMA
  descriptor is a contiguous 2KB run.
"""

import numpy as np
from collections import deque
from contextlib import ExitStack

S = 4096
D = 1024
NCORES = 8
HD = 128  # head-dim span per core (2 heads x 64)
DK = 64
NT = S // 128   # 32 t-blocks
NWV = 8         # s-waves of 512
LAG_W0 = 24   # wave-0 PV lag: pushes PV+v work past the k/q-proj burst
LAG = 2       # steady-state PV lag (lt(i+1) is emitted before the
              # pops, so a briefly-waiting PV never blocks logits)
LAG_LAST = 2  # final wave drains tight to shorten the tail
PE_BUDGET = 600   # max popped PE-ns per iteration (guards the exp cadence)
V_TRACK = True    # v-blocks emitted from inside wave-0 PV pops (+4 lookahead)
Q_MID = False     # q pieces at wave end (mid-wave collides with finalize)
LAG_W1 = 14       # wave-1 PV lag (wave-0 backlog still draining)

# Schraudolph exp-on-DVE: i16 = trunc/round(A*(0.125*lt) + B); bitcast bf16
# gives e^z * (1 + O(3%)).  A = 128*log2(e) folded with the 0.125 logit
# scale; B centers the linear-in-mantissa interpolation error (calibrated
# against the real convert rounding via test.py sweep).
SCH_A = float(128.0 / np.log(2.0) * 0.125)
SCH_B = 16256.0 - 5.0
# t-blocks whose exp runs on DVE (Schraudolph) instead of ACT, per wave.
# 12/32 offload in an A,A,D-ish weave: ACT does 20 exps/wave (~21us), DVE
# ~14us + evacuations; both below the PE wave time (~27.8us), making PE the
# pacing engine.  tb 0-3 stay on ACT so the wave-boundary recip/scale (the
# serial acc handoff, accpool bufs=1) isn't queued behind a 1.2us DVE exp.
DVE_TBS = {4, 6, 9, 11, 14, 16, 19, 21, 24, 26, 29, 31}
DVE_TBS_W0 = {14, 17, 20, 23, 26, 29}  # wave 0: DVE busy with v/proj backlog

_NC_CACHE = {}


def _split_multi_waits(nc, mybir):
    """This walrus build encodes at most ~2 sync commands per instruction
    (1 for matmul/drain). Keep <=1 wait on every compute/DMA instruction and
    move the rest into standalone dual-condition EventSemaphore instructions
    inserted immediately before it on the same engine (same wait point, so
    semantics are unchanged)."""
    n = 0
    used = set()
    for b in nc.m.functions[0].blocks:
        for inst in b.instructions:
            si = inst.sync_info
            if si:
                for w in (si.on_wait or []):
                    used.add(w.id)
                for u in (si.on_update or []):
                    used.add(u.id)
    free_ids = [i for i in range(max(used) + 1, max(used) + 32)]
    sems = {}

    def eng_sem(eng):
        if eng not in sems:
            sems[eng] = (free_ids.pop(0), f"wsplit_{len(sems)}")
        return sems[eng]

    for b in nc.m.functions[0].blocks:
        il = b.instructions
        new = []
        for inst in il:
            si = inst.sync_info
            waits = list(si.on_wait) if si and si.on_wait else []
            upds = list(si.on_update) if si and si.on_update else []
            if type(inst).__name__ == "InstEventSemaphore":
                new.append(inst)
                continue
            if len(waits) > 1:
                excess, keep = waits[:-1], waits[-1:]
                for i in range(0, len(excess), 2):
                    sid, sname = eng_sem(inst.engine)
                    ev = mybir.InstEventSemaphore(
                        name=f"{inst.name}_ws{i}", engine=inst.engine,
                        ins=[], outs=[],
                        sync_info=mybir.SyncInfo(
                            on_wait=excess[i:i + 2],
                            on_update=[mybir.SyncUpdate(
                                sync_type="semaphore", id=sid,
                                ant_name=sname, update_mode="sem-inc",
                                update_value=1, update_reg=None)]))
                    new.append(ev)
                    n += 1
                inst.sync_info = mybir.SyncInfo(on_wait=keep, on_update=upds)
            new.append(inst)
        il[:] = new
    return n


def _build_nc():
    import concourse.bass as bass
    import concourse.tile as tile
    import concourse.mybir as mybir

    fp32 = mybir.dt.float32
    bf16 = mybir.dt.bfloat16
    i16 = mybir.dt.int16
    AF = mybir.ActivationFunctionType
    ALU = mybir.AluOpType

    nc = bass.Bass()
    xT = nc.dram_tensor("xT", [D, S], bf16, kind="ExternalInput")
    # weights pre-laid host-side as [p, c, m] so each DMA descriptor is a
    # contiguous 2KB run (256B descriptors pay a 2x latency penalty)
    wqT = nc.dram_tensor("wqT", [128, 8 * HD], bf16, kind="ExternalInput")
    wkT = nc.dram_tensor("wkT", [128, 8 * HD], bf16, kind="ExternalInput")
    wvT = nc.dram_tensor("wvT", [128, 8 * HD], bf16, kind="ExternalInput")
    woT = nc.dram_tensor("woT", [HD, D], bf16, kind="ExternalInput")
    ident = nc.dram_tensor("ident", [128, 128], bf16, kind="ExternalInput")
    y = nc.dram_tensor("y", [S, D], bf16, kind="ExternalOutput")

    with tile.TileContext(nc) as tc, ExitStack() as ctx, \
         nc.allow_low_precision(reason="bf16 operands within rel-err budget"):
        sb = ctx.enter_context(tc.tile_pool(name="sb", bufs=1))
        qT_sb = sb.tile([128, S], bf16, tag="qT")
        kT_sb = sb.tile([128, S], bf16, tag="kT")
        # v_aug[:, tb, h, 0:64] = v block for head h; [..., 64] = ones
        v_aug = sb.tile([128, NT, 2, DK + 1], bf16, tag="vaug")
        wq_sb = sb.tile([128, 8, HD], bf16, tag="wq")
        wk_sb = sb.tile([128, 8, HD], bf16, tag="wk")
        wv_sb = sb.tile([128, 8, HD], bf16, tag="wv")
        wo_sb = sb.tile([HD, D], bf16, tag="wo")
        id_sb = sb.tile([128, 128], bf16, tag="id")
        warm = sb.tile([1, DK], fp32, tag="warm")
        wsrc = sb.tile([128, 512], bf16, tag="wsrc")

        xpool = ctx.enter_context(tc.tile_pool(name="xpool", bufs=8))
        ptpool = ctx.enter_context(tc.tile_pool(name="ptpool", bufs=LAG_W0 + 14))
        attnpool = ctx.enter_context(tc.tile_pool(name="attnpool", bufs=2))
        atTpool = ctx.enter_context(tc.tile_pool(name="atTpool", bufs=4))
        yopool = ctx.enter_context(tc.tile_pool(name="yopool", bufs=8))
        dpool = ctx.enter_context(tc.tile_pool(name="dpool", bufs=2))
        ltpool = ctx.enter_context(
            tc.tile_pool(name="ltpool", bufs=4, space="PSUM"))
        accpool = ctx.enter_context(
            tc.tile_pool(name="accpool", bufs=1, space="PSUM"))
        pspool = ctx.enter_context(
            tc.tile_pool(name="pspool", bufs=2, space="PSUM"))

        # ---- startup DMAs: critical path (wk, wq, x piece 0) first ----
        nc.sync.dma_start(wk_sb[:], wkT.rearrange("p (c m) -> p c m", c=8))
        nc.sync.dma_start(wq_sb[:], wqT.rearrange("p (c m) -> p c m", c=8))
        xTr = xT.rearrange("(c p) s -> p c s", p=128)
        xqs = {}

        def load_xq(p):
            xq = xpool.tile([128, 8, 512], bf16, tag="xq", name=f"xq_{p}")
            for c in range(0, 8, 2):
                nc.sync.dma_start(xq[:, c:c + 2, :],
                                  xTr[:, c:c + 2, p * 512:(p + 1) * 512])
            xqs[p] = xq

        load_xq(0)
        nc.sync.dma_start(wv_sb[:], wvT.rearrange("p (c m) -> p c m", c=8))
        nc.sync.dma_start(wo_sb[:], woT[:])
        nc.sync.dma_start(id_sb[:], ident[:])
        # Warm-up fed by an on-chip memset (no DMA dependency). Matmul cost
        # is priced at DECODE time, which runs ~36 instructions (exec-queue
        # depth) ahead of execution: to get the real projection matmuls
        # priced at the full-speed p-state, the PE must (a) be continuously
        # busy >3us and (b) have >=36 instructions in flight before them.
        # 8 big warms ramp the clock, 36 tiny ones flush the decode queue.
        # wsrc memset first: the warm matmuls wait on it.
        nc.vector.memset(wsrc[:], 0.5)
        nc.vector.memset(v_aug[:, :, :, DK], 1.0)
        nc.scalar.activation(warm[:], wsrc[0:1, 0:DK], AF.Exp, scale=0.125)
        warm_slots = [
            pspool.tile([128, 512], fp32, tag="ps", name=f"warmps_{i}")
            for i in range(2)]
        warm_lt = [
            ltpool.tile([128, 512], fp32, tag="lt", name=f"warmlt_{i}")
            for i in range(4)]
        for i in range(8):
            wp = (warm_slots[i % 2][:, 0:512] if i % 4 < 2
                  else warm_lt[i % 4][:])
            nc.tensor.matmul(wp, wsrc[:, 0:128], wsrc[:, 0:512],
                             start=True, stop=True)
        for i in range(36):
            wp = (warm_slots[i % 2][:, 0:32] if i % 4 < 2
                  else warm_lt[i % 4][:, 0:32])
            nc.tensor.matmul(wp, wsrc[:, 0:128], wsrc[:, 0:32],
                             start=True, stop=True)
        # remaining x pieces; all 8 stay resident (q-proj of piece p runs
        # ~4 waves after its k-proj, so slots can't rotate)
        for p in range(1, 8):
            load_xq(p)

        # ---- projection pieces (as resumable quarter thunks) ----
        def qk_proj_thunks(p, w_sb, dst):
            """Four thunks of 2 chunks each for dst[:, p*512:(p+1)*512]."""
            hold = {}

            def quarter(i):
                def run():
                    if i == 0:
                        hold["ps"] = pspool.tile(
                            [128, 512], fp32, tag="ps",
                            name=f"pqk_{dst.tensor.name}_{p}")
                    ps = hold["ps"]
                    for c in range(2 * i, 2 * i + 2):
                        nc.tensor.matmul(ps[:], w_sb[:, c, :],
                                         xqs[p][:, c, :], start=(c == 0),
                                         stop=(c == 7))
                    if i == 3:
                        nc.vector.tensor_copy(
                            dst[:, p * 512:(p + 1) * 512], hold.pop("ps")[:])
                return run
            return [quarter(i) for i in range(4)]

        def v_block_thunk(tb):
            def run():
                p, i = tb // 4, tb % 4
                vps = pspool.tile([128, 128], fp32, tag="ps",
                                  name=f"vps_{tb}")
                for c in range(8):
                    nc.tensor.matmul(
                        vps[:], xqs[p][:, c, i * 128:(i + 1) * 128],
                        wv_sb[:, c, :], start=(c == 0), stop=(c == 7))
                nc.vector.tensor_copy(
                    v_aug[:, tb, :, 0:DK],
                    vps[:].rearrange("p (h d) -> p h d", h=2))
            return run

        # ---- startup projections: piece 0 of q and k, interleaved per
        # x-chunk so matmuls start as soon as each chunk DMA lands; the
        # two PSUM->SBUF copies run on different engines in parallel ----
        pq0 = pspool.tile([128, 512], fp32, tag="ps", name="pq0")
        pk0 = pspool.tile([128, 512], fp32, tag="ps", name="pk0")
        xq0 = xqs[0]
        for c in range(8):
            nc.tensor.matmul(pq0[:], wq_sb[:, c, :], xq0[:, c, :],
                             start=(c == 0), stop=(c == 7))
            nc.tensor.matmul(pk0[:], wk_sb[:, c, :], xq0[:, c, :],
                             start=(c == 0), stop=(c == 7))
        nc.scalar.copy(qT_sb[:, 0:512], pq0[:])
        # lt(0,0) needs only k block 0: give it its own small copy
        nc.vector.tensor_copy(kT_sb[:, 0:128], pk0[:, 0:128])
        nc.vector.tensor_copy(kT_sb[:, 128:512], pk0[:, 128:512])

        # deadline-ordered weave of the remaining projection work; each
        # entry is [deadline, pe_cost_ns, thunk]
        proj_q = deque()
        for p in range(1, 8):
            for j, t in enumerate(qk_proj_thunks(p, wk_sb, kT_sb)):
                proj_q.append([4 * p - 5 + j, 427, t])
        # v blocks 0..3 by deadline; the rest are emitted from inside the
        # wave-0 PV pops with a +4 lookahead, auto-tracking the drain rate
        v_done = set()

        def ensure_v(tb):
            if tb < NT and tb not in v_done:
                v_done.add(tb)
                v_block_thunk(tb)()

        n_pre_v = 4 if V_TRACK else NT
        for tb in range(n_pre_v):
            proj_q.append([LAG_W0 - 6 + tb, 427, lambda tb=tb: ensure_v(tb)])
        # q piece w must be fully copied before lt(w, 0) is emitted at
        # gi 32w-1: the last quarter lands at base+6 <= 32w-3
        for j, t in enumerate(qk_proj_thunks(1, wq_sb, qT_sb)):
            proj_q.append([(16 if Q_MID else 22) + 2 * j, 427, t])
        for w in range(2, NWV):
            for j, t in enumerate(qk_proj_thunks(w, wq_sb, qT_sb)):
                base = 32 * (w - 1) + (10 if Q_MID else 22)
                proj_q.append([base + 2 * j, 427, t])
        proj_q = deque(sorted(proj_q, key=lambda e: e[0]))

        # ---- attention machinery ----
        # pending: [floor, cost, kind, wave, thunk]; kind "pv" | "sc" | "fin".
        # Ordering gates (enforced at pop time, not just via floors):
        #   fin/sc(w) pop only after all pv(w) popped;
        #   pv(w+1) pops only after all sc(w) popped (acc slot handoff).
        pending = []
        live_pv = {w: 0 for w in range(NWV)}
        live_sc = {w: 0 for w in range(-1, NWV)}
        live_sc[-1] = 0
        state = {"gi": 0}
        lt_holder = {}

        def emit_lt(w, tb):
            # per-head lt tiles (1 PSUM bank each): each head's
            # lt -> exp -> slot-free chain ping-pongs independently across
            # 2 of the 4 slots, halving the latency the cadence must absorb
            s0 = w * 512
            lts = []
            for h in range(2):
                lt_h = ltpool.tile([128, 512], fp32, tag="lt",
                                   name=f"lt_{w}_{tb}_{h}")
                nc.tensor.matmul(
                    lt_h[:],
                    kT_sb[DK * h:DK * (h + 1), tb * 128:(tb + 1) * 128],
                    qT_sb[DK * h:DK * (h + 1), s0:s0 + 512],
                    start=True, stop=True,
                    tile_position=(DK * h, 0),
                )
                lts.append(lt_h)
            return lts

        def pv_thunk(w, tb, pt, accs):
            def run():
                if w == 0 and V_TRACK:
                    ensure_v(tb)
                    ensure_v(tb + 4)
                if tb == 0:
                    accs.append(accpool.tile([128, 2, 4, 128], fp32,
                                             tag="acc", name=f"acc_{w}"))
                acc = accs[0]
                p_all = (pt[:].bitcast(bf16) if pt.tensor.dtype == i16
                         else pt[:])
                for h in range(2):
                    for k in range(4):
                        # one accumulation group per PSUM zero-region (the
                        # 2KB bank holding all 4 k-slices of head h): start
                        # marks the whole bank pending-zero, so only the
                        # first slice may start and only the last may stop;
                        # the other tb==0 writes zero-on-first-touch.
                        nc.tensor.matmul(
                            acc[:, h, k, 0:DK + 1],
                            p_all[:, h * 512 + k * 128:h * 512 + (k + 1) * 128],
                            v_aug[:, tb, h, :],
                            start=(tb == 0 and k == 0),
                            stop=(tb == NT - 1 and k == 3),
                            skip_group_check=(k != 0),
                        )
            return run

        def finalize_thunks(w, accs):
            rden = dpool.tile([128, 2, 4], fp32, tag="rden", name=f"rden_{w}")
            attn_all = attnpool.tile([128, 2, 4, DK], bf16, tag="attn",
                                     name=f"attn_{w}")
            atT = {}

            def recip():
                nc.vector.reciprocal(rden[:], accs[0][:, :, :, DK])

            def scale_ks(k0, k1):
                # one DVE op normalizes all (h, k0:k1) blocks: rden broadcast
                # along dk via a stride-0 free dim
                nc.vector.tensor_tensor(
                    attn_all[:, :, k0:k1, :], accs[0][:, :, k0:k1, 0:DK],
                    rden[:, :, k0:k1, None].to_broadcast([128, 2, k1 - k0, DK]),
                    ALU.mult)

            def scale_act(k):
                # last wave: late blocks on the otherwise-idle ACT so their
                # transposes unblock without waiting out DVE's queue
                for h in range(2):
                    nc.scalar.activation(attn_all[:, h, k, :],
                                         accs[0][:, h, k, 0:DK],
                                         AF.Copy, scale=rden[:, h, k:k + 1])

            last = w == NWV - 1

            def transp(k):
                # both heads transpose into one [128,128] psum tile (disjoint
                # partition ranges -> no zero-region conflict), one copy out
                atT[k] = atTpool.tile([128, 128], bf16, tag="atT",
                                      name=f"atT_{w}_{k}")
                tps = pspool.tile([128, 128], bf16, tag="ps",
                                  name=f"tps_{w}_{k}")
                for h in range(2):
                    nc.tensor.transpose(tps[DK * h:DK * (h + 1), :],
                                        attn_all[:, h, k, :], id_sb[:])
                nc.vector.tensor_copy(atT[k][:], tps[:])

            def yblock(k):
                # one combined [128,1024] output + ONE y DMA per s-block:
                # HWDGE serializes issues at ~625ns, so halving the DMA
                # count shortens the end-of-kernel trickle
                b = w * 4 + k
                yo = yopool.tile([128, 1024], bf16, tag="yo",
                                 name=f"yo_{b}")
                if last:
                    # the lt slots are dead after the final exp and the acc
                    # banks after the scales: draw per-half yp banks from
                    # them so the tail pipelines 4+ deep, and split the
                    # copies across the idle ACT and DVE
                    if k == 1:
                        ypt = accpool.tile([128, 2, 512], fp32, tag="acc",
                                           name=f"yp_{b}")
                        yps = [ypt[:, jc, :] for jc in range(2)]
                    else:
                        yps = [ltpool.tile([128, 512], fp32, tag="lt",
                                           name=f"yp_{b}_{jc}")[:]
                               for jc in range(2)]
                    for jc in range(2):
                        nc.tensor.matmul(
                            yps[jc], atT[k][:],
                            wo_sb[:, jc * 512:(jc + 1) * 512],
                            start=True, stop=True)
                    nc.scalar.copy(yo[:, 0:512], yps[0])
                    nc.vector.tensor_copy(yo[:, 512:1024], yps[1])
                else:
                    for jc in range(2):
                        yp = pspool.tile([128, 512], fp32, tag="ps",
                                         name=f"yp_{b}_{jc}")
                        nc.tensor.matmul(
                            yp[:], atT[k][:],
                            wo_sb[:, jc * 512:(jc + 1) * 512],
                            start=True, stop=True)
                        # split the two copies across ACT/DVE for balance
                        if jc == 0:
                            nc.scalar.copy(yo[:, 0:512], yp[:])
                        else:
                            nc.vector.tensor_copy(
                                yo[:, jc * 512:(jc + 1) * 512], yp[:])
                nc.sync.dma_start(y[b * 128:(b + 1) * 128, :], yo[:])

            # (floor_offset, pe_cost_ns, thunk): scales all run first (they
            # are acc's only readers, so the next wave's PV start unblocks
            # early); transposes and output blocks then trickle so the
            # finalize never oversubscribes an iteration's PE slack
            thunks = [(1, 10, "sc", "dve", recip)]
            if last:
                thunks.append((2, 10, "sc", "dve", lambda: scale_ks(0, 2)))
                thunks.append((3, 10, "sc", "act", lambda: scale_act(2)))
                thunks.append((4, 10, "sc", "act", lambda: scale_act(3)))
            else:
                thunks.append((2, 10, "sc", "dve", lambda: scale_ks(0, 4)))
            for k in range(4):
                thunks.append((5 + 6 * k, 110, "fin", "dve",
                               lambda k=k: transp(k)))
                thunks.append((7 + 6 * k, 430, "fin", None,
                               lambda k=k: yblock(k)))
            return thunks

        def emit_iter(w, tb, accs):
            gi = state["gi"]
            lag = {0: LAG_W0, 1: LAG_W1, NWV - 1: LAG_LAST}.get(w, LAG)
            if w >= 2 and w != NWV - 1:
                # smooth the wave-boundary acc handoff (PVlast(w-1) -> recip
                # -> scale -> PV0(w), accpool bufs=1): give the first PVs of
                # the wave extra lag so the serial chain hides under lt/proj
                # work instead of stalling the PE
                lag = max(LAG, 6 - tb)
            lt = lt_holder.pop("lt")
            dve_tbs = DVE_TBS_W0 if w == 0 else DVE_TBS
            # exp in per-head halves: the lt slot's h0 half frees one
            # exp-half earlier, cutting the lt->exp->slot-free round trip
            # below the PE iteration time (the slot ping-pong is only 2 deep;
            # PSUM can't fit 3) — subtile deps let lt(i+2)'s h0 matmul start
            # as soon as exp(i)'s h0 half completes
            if tb in dve_tbs:
                # Schraudolph exp on DVE: affine into int16, bitcast bf16
                pt = ptpool.tile([128, 1024], i16, tag="pt",
                                 name=f"pt_{w}_{tb}")
                for hh in range(2):
                    nc.vector.tensor_scalar(
                        pt[:, hh * 512:(hh + 1) * 512],
                        lt[hh][:], SCH_A, SCH_B,
                        ALU.mult, ALU.add)
            else:
                pt = ptpool.tile([128, 1024], bf16, tag="pt",
                                 name=f"pt_{w}_{tb}")
                for hh in range(2):
                    nc.scalar.activation(pt[:, hh * 512:(hh + 1) * 512],
                                         lt[hh][:],
                                         AF.Exp, scale=0.125)
            if tb + 1 < NT:
                lt_holder["lt"] = emit_lt(w, tb + 1)
            elif w + 1 < NWV:
                lt_holder["lt"] = emit_lt(w + 1, 0)
            live_pv[w] += 1
            cur_eng = "dve" if tb in dve_tbs else "act"
            pending.append([gi + lag, 644 if (w == 0 and V_TRACK) else 217,
                            "pv", None, w, pv_thunk(w, tb, pt, accs)])
            # Pop READY items anywhere in the list (a far-future finalize
            # floor must not head-block the PV stream), but cap the popped
            # PE-ns per iteration, and defer evacuation thunks whose engine
            # just received this iteration's exp (a copy queued between exp
            # halves delays the lt slot free and stalls the PE).
            budget = PE_BUDGET
            i = 0
            while i < len(pending):
                floor, cost, kind, eng, wv, t = pending[i]
                ok = floor <= gi and (cost <= budget or floor <= gi - 12)
                if ok and eng == cur_eng and floor > gi - 6:
                    ok = False
                if ok and kind == "pv":
                    ok = live_sc[wv - 1] == 0
                elif ok:
                    ok = live_pv[wv] == 0
                if ok:
                    pending.pop(i)
                    t()
                    budget -= cost
                    if kind == "pv":
                        live_pv[wv] -= 1
                    elif kind == "sc":
                        live_sc[wv] -= 1
                else:
                    i += 1
            # projection deadlines are HARD (logits read qT/kT at fixed
            # iterations): pop regardless of remaining budget
            pops = 0
            while proj_q and proj_q[0][0] <= gi and pops < 2:
                proj_q.popleft()[2]()
                pops += 1
            state["gi"] = gi + 1

        # ---- main loop ----
        lt_holder["lt"] = emit_lt(0, 0)
        for w in range(NWV):
            accs = []
            for tb in range(NT):
                emit_iter(w, tb, accs)
            for off, cost, kind, eng, t in finalize_thunks(w, accs):
                if kind == "sc":
                    live_sc[w] += 1
                pending.append([state["gi"] + LAG - 2 + off, cost, kind,
                                eng, w, t])
        while proj_q:
            proj_q.popleft()[2]()
        # final drain: keep list order within a wave; gates are satisfied
        # by construction (pv entries precede sc precede fin per wave)
        for e in pending:
            e[5]()

    _split_multi_waits(nc, mybir)
    nc.finalize()
    return nc


def _get_nc():
    if "nc" not in _NC_CACHE:
        _NC_CACHE["nc"] = _build_nc()
    return _NC_CACHE["nc"]


def _relay(wT):
    """[1024 d, 128 m] -> [p, c*m] with wT[c*128+p, m] at [p, c, m]: every
    DMA descriptor becomes a contiguous 2KB run."""
    return np.ascontiguousarray(
        wT.reshape(8, 128, HD).transpose(1, 0, 2).reshape(128, 8 * HD))


def _in_maps(x, Wq, Wk, Wv, Wo):
    import ml_dtypes
    bf16 = ml_dtypes.bfloat16
    xT = np.ascontiguousarray(np.asarray(x, np.float32).T).astype(bf16)
    ident = np.eye(128, dtype=np.float32).astype(bf16)
    maps = []
    for c in range(NCORES):
        sl = slice(HD * c, HD * (c + 1))
        maps.append(dict(
            xT=xT,
            wqT=_relay(np.asarray(Wq)[sl, :].T.astype(bf16)),
            wkT=_relay(np.asarray(Wk)[sl, :].T.astype(bf16)),
            wvT=_relay(np.asarray(Wv)[sl, :].T.astype(bf16)),
            woT=np.ascontiguousarray(np.asarray(Wo)[:, sl].T).astype(bf16),
            ident=ident,
        ))
    return maps


def kernel(x, Wq, Wk, Wv, Wo):
    from concourse.bass_utils import run_bass_kernel_spmd

    x = np.asarray(x, dtype=np.float32)
    nc = _get_nc()
    res = run_bass_kernel_spmd(nc, _in_maps(x, Wq, Wk, Wv, Wo),
                               list(range(NCORES)))
    out = np.zeros((S, D), np.float32)
    for rr in res.results:
        out += np.asarray(rr["y"], dtype=np.float32)
    return out



# revision 36
# speedup vs baseline: 1.3009x; 1.0260x over previous
"""Multi-head attention (S=4096, D=1024, H=16) on 8 trn2 NeuronCores.

Sharding: 2 heads per core (tensor-parallel on Q/K/V column splits and
dense row split). Each core computes a partial [S, D] output; host sums
the 8 partials (the unshard step for row-parallel TP).

Per-core design (bf16 operands, fp32 PSUM accumulate):
  The ACT engine's exp over the S*S*2 logits (256 x [128,1024] tiles,
  ~1.04us each) is the hard floor (~266us busy); everything else is
  scheduled to hide inside it (this build: ~292us total = 10.8us
  startup + 266us exp + ~2.5us exp stalls + ~12.4us finalize tail).
  - logits: lt[128t, 2h*512s] = k-block^T q-window per head (2 matmuls,
    512 rows each).
  - PV streams v, not P: stationary = pt s-block [128t,128s], moving =
    v_aug[128t,65] -> acc[128s,65] accumulated over 32 t-blocks (65
    rows/matmul instead of 512); column 64 of v_aug is ones so the
    softmax denominator lands in acc[...,64] already laid out per
    s-partition. One accumulation group per PSUM 2KB zero-region: only
    (tb0,k0) starts and (tb31,k3) stops each head's bank.
  - normalization: reciprocal of the denominator, then per-partition
    tensor_scalar_mul fused into the acc->SBUF copy.
  - v-projection emits v directly in [s-part, dk] layout (stationary =
    x-block, moving = Wv chunk): no PE transposes of V.
  - output projection: per s-block, transpose normalized attn
    ([128s,64]->[64dk,128s] via PE, 1 cyc/row in bf16), then
    yp[128s,512d] = attnT^T wo; y is bf16, host sums partials in fp32.
  PSUM (8 banks): lt 2x2 (ping-pong), acc 2 (padded [128,2,4,128],
  single-buffered across waves), 2-slot scratch ring shared by proj
  psum / transposes / yp; the last wave's yp also rotates through the
  then-dead lt+acc banks so the tail pipelines 3-deep.
  Schedule: one exp per iteration paces everything. Between consecutive
  lt emissions the PE tolerates only ~1.2us of other work, so PV /
  finalize thunks pop from a floor-gated list under a PE-ns budget;
  projection deadlines are hard (logits read qT/kT at fixed iterations)
  and ordering gates enforce pv(w) < scales(w) < pv(w+1) (acc handoff).
  GPSIMD cannot touch PSUM, so all PSUM evacuation is DVE/ACT.
  Startup: matmul cost is priced at decode time ~36 instructions ahead
  of execution, so 8 big + 36 tiny memset-fed warm-up matmuls both ramp
  the PE p-state and flush the decode queue before the first real
  projections; DMA order puts wk/wq/x-piece-0 first (HWDGE serializes
  at ~625ns per dma_start) and weights are host-re-laid so every DMA
  descriptor is a contiguous 2KB run.
"""

import numpy as np
from collections import deque
from contextlib import ExitStack

S = 4096
D = 1024
NCORES = 8
HD = 128  # head-dim span per core (2 heads x 64)
DK = 64
NT = S // 128   # 32 t-blocks
NWV = 8         # s-waves of 512
LAG_W0 = 24   # wave-0 PV lag: pushes PV+v work past the k/q-proj burst
LAG = 2       # steady-state PV lag (lt(i+1) is emitted before the
              # pops, so a briefly-waiting PV never blocks logits)
LAG_LAST = 2  # final wave drains tight to shorten the tail
PE_BUDGET = 600   # max popped PE-ns per iteration (guards the exp cadence)
V_TRACK = True    # v-blocks emitted from inside wave-0 PV pops (+4 lookahead)
Q_MID = False     # q pieces at wave end (mid-wave collides with finalize)
LAG_W1 = 14       # wave-1 PV lag (wave-0 backlog still draining)

# Schraudolph exp-on-DVE: i16 = trunc/round(A*(0.125*lt) + B); bitcast bf16
# gives e^z * (1 + O(3%)).  A = 128*log2(e) folded with the 0.125 logit
# scale; B centers the linear-in-mantissa interpolation error (calibrated
# against the real convert rounding via test.py sweep).
SCH_A = float(128.0 / np.log(2.0) * 0.125)
SCH_B = 16256.0 - 5.0
# t-blocks whose exp runs on DVE (Schraudolph) instead of ACT, per wave.
# 12/32 offload in an A,A,D-ish weave: ACT does 20 exps/wave (~21us), DVE
# ~14us + evacuations; both below the PE wave time (~27.8us), making PE the
# pacing engine.  tb 0-3 stay on ACT so the wave-boundary recip/scale (the
# serial acc handoff, accpool bufs=1) isn't queued behind a 1.2us DVE exp.
DVE_TBS = {1, 3, 6, 8, 11, 13, 16, 18, 21, 23, 26, 28, 31}
DVE_TBS_W0 = {14, 17, 20, 23, 26, 29}  # wave 0: DVE busy with v/proj backlog

import os as _os
if _os.environ.get("KCFG_DVE_TBS"):
    DVE_TBS = set(int(x) for x in _os.environ["KCFG_DVE_TBS"].split(","))
if _os.environ.get("KCFG_DVE_TBS_W0"):
    DVE_TBS_W0 = set(int(x) for x in _os.environ["KCFG_DVE_TBS_W0"].split(","))
if _os.environ.get("KCFG_SCH_B"):
    SCH_B = float(_os.environ["KCFG_SCH_B"])

_NC_CACHE = {}


def _split_multi_waits(nc, mybir):
    """This walrus build encodes at most ~2 sync commands per instruction
    (1 for matmul/drain). Keep <=1 wait on every compute/DMA instruction and
    move the rest into standalone dual-condition EventSemaphore instructions
    inserted immediately before it on the same engine (same wait point, so
    semantics are unchanged)."""
    n = 0
    used = set()
    for b in nc.m.functions[0].blocks:
        for inst in b.instructions:
            si = inst.sync_info
            if si:
                for w in (si.on_wait or []):
                    used.add(w.id)
                for u in (si.on_update or []):
                    used.add(u.id)
    free_ids = [i for i in range(max(used) + 1, max(used) + 32)]
    sems = {}

    def eng_sem(eng):
        if eng not in sems:
            sems[eng] = (free_ids.pop(0), f"wsplit_{len(sems)}")
        return sems[eng]

    for b in nc.m.functions[0].blocks:
        il = b.instructions
        new = []
        for inst in il:
            si = inst.sync_info
            waits = list(si.on_wait) if si and si.on_wait else []
            upds = list(si.on_update) if si and si.on_update else []
            if type(inst).__name__ == "InstEventSemaphore":
                new.append(inst)
                continue
            if len(waits) > 1:
                excess, keep = waits[:-1], waits[-1:]
                for i in range(0, len(excess), 2):
                    sid, sname = eng_sem(inst.engine)
                    ev = mybir.InstEventSemaphore(
                        name=f"{inst.name}_ws{i}", engine=inst.engine,
                        ins=[], outs=[],
                        sync_info=mybir.SyncInfo(
                            on_wait=excess[i:i + 2],
                            on_update=[mybir.SyncUpdate(
                                sync_type="semaphore", id=sid,
                                ant_name=sname, update_mode="sem-inc",
                                update_value=1, update_reg=None)]))
                    new.append(ev)
                    n += 1
                inst.sync_info = mybir.SyncInfo(on_wait=keep, on_update=upds)
            new.append(inst)
        il[:] = new
    return n


def _build_nc():
    import concourse.bass as bass
    import concourse.tile as tile
    import concourse.mybir as mybir

    fp32 = mybir.dt.float32
    bf16 = mybir.dt.bfloat16
    i16 = mybir.dt.int16
    AF = mybir.ActivationFunctionType
    ALU = mybir.AluOpType

    nc = bass.Bass()
    xT = nc.dram_tensor("xT", [D, S], bf16, kind="ExternalInput")
    # weights pre-laid host-side as [p, c, m] so each DMA descriptor is a
    # contiguous 2KB run (256B descriptors pay a 2x latency penalty)
    wqT = nc.dram_tensor("wqT", [128, 8 * HD], bf16, kind="ExternalInput")
    wkT = nc.dram_tensor("wkT", [128, 8 * HD], bf16, kind="ExternalInput")
    wvT = nc.dram_tensor("wvT", [128, 8 * HD], bf16, kind="ExternalInput")
    woT = nc.dram_tensor("woT", [HD, D], bf16, kind="ExternalInput")
    ident = nc.dram_tensor("ident", [128, 128], bf16, kind="ExternalInput")
    y = nc.dram_tensor("y", [S, D], bf16, kind="ExternalOutput")

    with tile.TileContext(nc) as tc, ExitStack() as ctx, \
         nc.allow_low_precision(reason="bf16 operands within rel-err budget"):
        sb = ctx.enter_context(tc.tile_pool(name="sb", bufs=1))
        qT_sb = sb.tile([128, S], bf16, tag="qT")
        kT_sb = sb.tile([128, S], bf16, tag="kT")
        # v_aug[:, tb, h, 0:64] = v block for head h; [..., 64] = ones
        v_aug = sb.tile([128, NT, 2, DK + 1], bf16, tag="vaug")
        wq_sb = sb.tile([128, 8, HD], bf16, tag="wq")
        wk_sb = sb.tile([128, 8, HD], bf16, tag="wk")
        wv_sb = sb.tile([128, 8, HD], bf16, tag="wv")
        wo_sb = sb.tile([HD, D], bf16, tag="wo")
        id_sb = sb.tile([128, 128], bf16, tag="id")
        warm = sb.tile([1, DK], fp32, tag="warm")
        wsrc = sb.tile([128, 512], bf16, tag="wsrc")

        xpool = ctx.enter_context(tc.tile_pool(name="xpool", bufs=8))
        ptpool = ctx.enter_context(tc.tile_pool(name="ptpool", bufs=LAG_W0 + 14))
        attnpool = ctx.enter_context(tc.tile_pool(name="attnpool", bufs=2))
        atTpool = ctx.enter_context(tc.tile_pool(name="atTpool", bufs=4))
        yopool = ctx.enter_context(tc.tile_pool(name="yopool", bufs=8))
        dpool = ctx.enter_context(tc.tile_pool(name="dpool", bufs=2))
        ltpool = ctx.enter_context(
            tc.tile_pool(name="ltpool", bufs=4, space="PSUM"))
        accpool = ctx.enter_context(
            tc.tile_pool(name="accpool", bufs=1, space="PSUM"))
        pspool = ctx.enter_context(
            tc.tile_pool(name="pspool", bufs=2, space="PSUM"))

        # ---- startup DMAs: critical path (wk, wq, x piece 0) first ----
        nc.sync.dma_start(wk_sb[:], wkT.rearrange("p (c m) -> p c m", c=8))
        nc.sync.dma_start(wq_sb[:], wqT.rearrange("p (c m) -> p c m", c=8))
        xTr = xT.rearrange("(c p) s -> p c s", p=128)
        xqs = {}

        def load_xq(p):
            xq = xpool.tile([128, 8, 512], bf16, tag="xq", name=f"xq_{p}")
            for c in range(0, 8, 2):
                nc.sync.dma_start(xq[:, c:c + 2, :],
                                  xTr[:, c:c + 2, p * 512:(p + 1) * 512])
            xqs[p] = xq

        load_xq(0)
        nc.sync.dma_start(wv_sb[:], wvT.rearrange("p (c m) -> p c m", c=8))
        nc.sync.dma_start(wo_sb[:], woT[:])
        nc.sync.dma_start(id_sb[:], ident[:])
        # Warm-up fed by an on-chip memset (no DMA dependency). Matmul cost
        # is priced at DECODE time, which runs ~36 instructions (exec-queue
        # depth) ahead of execution: to get the real projection matmuls
        # priced at the full-speed p-state, the PE must (a) be continuously
        # busy >3us and (b) have >=36 instructions in flight before them.
        # 8 big warms ramp the clock, 36 tiny ones flush the decode queue.
        # wsrc memset first: the warm matmuls wait on it.
        nc.vector.memset(wsrc[:], 0.5)
        nc.vector.memset(v_aug[:, :, :, DK], 1.0)
        nc.scalar.activation(warm[:], wsrc[0:1, 0:DK], AF.Exp, scale=0.125)
        warm_slots = [
            pspool.tile([128, 512], fp32, tag="ps", name=f"warmps_{i}")
            for i in range(2)]
        warm_lt = [
            ltpool.tile([128, 512], fp32, tag="lt", name=f"warmlt_{i}")
            for i in range(4)]
        for i in range(8):
            wp = (warm_slots[i % 2][:, 0:512] if i % 4 < 2
                  else warm_lt[i % 4][:])
            nc.tensor.matmul(wp, wsrc[:, 0:128], wsrc[:, 0:512],
                             start=True, stop=True)
        for i in range(36):
            wp = (warm_slots[i % 2][:, 0:32] if i % 4 < 2
                  else warm_lt[i % 4][:, 0:32])
            nc.tensor.matmul(wp, wsrc[:, 0:128], wsrc[:, 0:32],
                             start=True, stop=True)
        # remaining x pieces; all 8 stay resident (q-proj of piece p runs
        # ~4 waves after its k-proj, so slots can't rotate)
        for p in range(1, 8):
            load_xq(p)

        # ---- projection pieces (as resumable quarter thunks) ----
        def qk_proj_thunks(p, w_sb, dst):
            """Four thunks of 2 chunks each for dst[:, p*512:(p+1)*512]."""
            hold = {}

            def quarter(i):
                def run():
                    if i == 0:
                        hold["ps"] = pspool.tile(
                            [128, 512], fp32, tag="ps",
                            name=f"pqk_{dst.tensor.name}_{p}")
                    ps = hold["ps"]
                    for c in range(2 * i, 2 * i + 2):
                        nc.tensor.matmul(ps[:], w_sb[:, c, :],
                                         xqs[p][:, c, :], start=(c == 0),
                                         stop=(c == 7))
                    if i == 3:
                        nc.vector.tensor_copy(
                            dst[:, p * 512:(p + 1) * 512], hold.pop("ps")[:])
                return run
            return [quarter(i) for i in range(4)]

        def v_block_thunk(tb):
            def run():
                p, i = tb // 4, tb % 4
                vps = pspool.tile([128, 128], fp32, tag="ps",
                                  name=f"vps_{tb}")
                for c in range(8):
                    nc.tensor.matmul(
                        vps[:], xqs[p][:, c, i * 128:(i + 1) * 128],
                        wv_sb[:, c, :], start=(c == 0), stop=(c == 7))
                nc.vector.tensor_copy(
                    v_aug[:, tb, :, 0:DK],
                    vps[:].rearrange("p (h d) -> p h d", h=2))
            return run

        # ---- startup projections: piece 0 of q and k, interleaved per
        # x-chunk so matmuls start as soon as each chunk DMA lands; the
        # two PSUM->SBUF copies run on different engines in parallel ----
        pq0 = pspool.tile([128, 512], fp32, tag="ps", name="pq0")
        pk0 = pspool.tile([128, 512], fp32, tag="ps", name="pk0")
        xq0 = xqs[0]
        for c in range(8):
            nc.tensor.matmul(pq0[:], wq_sb[:, c, :], xq0[:, c, :],
                             start=(c == 0), stop=(c == 7))
            nc.tensor.matmul(pk0[:], wk_sb[:, c, :], xq0[:, c, :],
                             start=(c == 0), stop=(c == 7))
        nc.scalar.copy(qT_sb[:, 0:512], pq0[:])
        # lt(0,0) needs only k block 0: give it its own small copy
        nc.vector.tensor_copy(kT_sb[:, 0:128], pk0[:, 0:128])
        nc.vector.tensor_copy(kT_sb[:, 128:512], pk0[:, 128:512])

        # deadline-ordered weave of the remaining projection work; each
        # entry is [deadline, pe_cost_ns, thunk]
        proj_q = deque()
        for p in range(1, 8):
            for j, t in enumerate(qk_proj_thunks(p, wk_sb, kT_sb)):
                proj_q.append([4 * p - 5 + j, 427, t])
        # v blocks 0..3 by deadline; the rest are emitted from inside the
        # wave-0 PV pops with a +4 lookahead, auto-tracking the drain rate
        v_done = set()

        def ensure_v(tb):
            if tb < NT and tb not in v_done:
                v_done.add(tb)
                v_block_thunk(tb)()

        n_pre_v = 4 if V_TRACK else NT
        for tb in range(n_pre_v):
            proj_q.append([LAG_W0 - 6 + tb, 427, lambda tb=tb: ensure_v(tb)])
        # q piece w must be fully copied before lt(w, 0) is emitted at
        # gi 32w-1: the last quarter lands at base+6 <= 32w-3
        for j, t in enumerate(qk_proj_thunks(1, wq_sb, qT_sb)):
            proj_q.append([(16 if Q_MID else 22) + 2 * j, 427, t])
        for w in range(2, NWV):
            for j, t in enumerate(qk_proj_thunks(w, wq_sb, qT_sb)):
                base = 32 * (w - 1) + (10 if Q_MID else 22)
                proj_q.append([base + 2 * j, 427, t])
        proj_q = deque(sorted(proj_q, key=lambda e: e[0]))

        # ---- attention machinery ----
        # pending: [floor, cost, kind, wave, thunk]; kind "pv" | "sc" | "fin".
        # Ordering gates (enforced at pop time, not just via floors):
        #   fin/sc(w) pop only after all pv(w) popped;
        #   pv(w+1) pops only after all sc(w) popped (acc slot handoff).
        pending = []
        live_pv = {w: 0 for w in range(NWV)}
        live_sc = {w: 0 for w in range(-1, NWV)}
        live_sc[-1] = 0
        state = {"gi": 0}
        lt_holder = {}

        def emit_lt(w, tb):
            # per-head lt tiles (1 PSUM bank each): each head's
            # lt -> exp -> slot-free chain ping-pongs independently across
            # 2 of the 4 slots, halving the latency the cadence must absorb
            s0 = w * 512
            lts = []
            for h in range(2):
                lt_h = ltpool.tile([128, 512], fp32, tag="lt",
                                   name=f"lt_{w}_{tb}_{h}")
                nc.tensor.matmul(
                    lt_h[:],
                    kT_sb[DK * h:DK * (h + 1), tb * 128:(tb + 1) * 128],
                    qT_sb[DK * h:DK * (h + 1), s0:s0 + 512],
                    start=True, stop=True,
                    tile_position=(DK * h, 0),
                )
                lts.append(lt_h)
            return lts

        def pv_thunk(w, tb, pt, accs):
            def run():
                if w == 0 and V_TRACK:
                    ensure_v(tb)
                    ensure_v(tb + 4)
                if tb == 0:
                    accs.append(accpool.tile([128, 2, 4, 128], fp32,
                                             tag="acc", name=f"acc_{w}"))
                acc = accs[0]
                p_all = (pt[:].bitcast(bf16) if pt.tensor.dtype == i16
                         else pt[:])
                for h in range(2):
                    for k in range(4):
                        # one accumulation group per PSUM zero-region (the
                        # 2KB bank holding all 4 k-slices of head h): start
                        # marks the whole bank pending-zero, so only the
                        # first slice may start and only the last may stop;
                        # the other tb==0 writes zero-on-first-touch.
                        nc.tensor.matmul(
                            acc[:, h, k, 0:DK + 1],
                            p_all[:, h * 512 + k * 128:h * 512 + (k + 1) * 128],
                            v_aug[:, tb, h, :],
                            start=(tb == 0 and k == 0),
                            stop=(tb == NT - 1 and k == 3),
                            skip_group_check=(k != 0),
                        )
            return run

        def finalize_thunks(w, accs):
            rden = dpool.tile([128, 2, 4], fp32, tag="rden", name=f"rden_{w}")
            attn_all = attnpool.tile([128, 2, 4, DK], bf16, tag="attn",
                                     name=f"attn_{w}")
            atT = {}

            def recip():
                nc.vector.reciprocal(rden[:], accs[0][:, :, :, DK])

            def scale_ks(k0, k1):
                # one DVE op normalizes all (h, k0:k1) blocks: rden broadcast
                # along dk via a stride-0 free dim
                nc.vector.tensor_tensor(
                    attn_all[:, :, k0:k1, :], accs[0][:, :, k0:k1, 0:DK],
                    rden[:, :, k0:k1, None].to_broadcast([128, 2, k1 - k0, DK]),
                    ALU.mult)

            def scale_act(k):
                # last wave: late blocks on the otherwise-idle ACT so their
                # transposes unblock without waiting out DVE's queue
                for h in range(2):
                    nc.scalar.activation(attn_all[:, h, k, :],
                                         accs[0][:, h, k, 0:DK],
                                         AF.Copy, scale=rden[:, h, k:k + 1])

            last = w == NWV - 1

            def transp(k):
                # both heads transpose into one [128,128] psum tile (disjoint
                # partition ranges -> no zero-region conflict), one copy out
                atT[k] = atTpool.tile([128, 128], bf16, tag="atT",
                                      name=f"atT_{w}_{k}")
                tps = pspool.tile([128, 128], bf16, tag="ps",
                                  name=f"tps_{w}_{k}")
                for h in range(2):
                    nc.tensor.transpose(tps[DK * h:DK * (h + 1), :],
                                        attn_all[:, h, k, :], id_sb[:])
                nc.vector.tensor_copy(atT[k][:], tps[:])

            def yblock(k):
                # one combined [128,1024] output + ONE y DMA per s-block:
                # HWDGE serializes issues at ~625ns, so halving the DMA
                # count shortens the end-of-kernel trickle
                b = w * 4 + k
                yo = yopool.tile([128, 1024], bf16, tag="yo",
                                 name=f"yo_{b}")
                if last:
                    # the lt slots are dead after the final exp and the acc
                    # banks after the scales: draw per-half yp banks from
                    # them so the tail pipelines 4+ deep, and split the
                    # copies across the idle ACT and DVE
                    if k == 1:
                        ypt = accpool.tile([128, 2, 512], fp32, tag="acc",
                                           name=f"yp_{b}")
                        yps = [ypt[:, jc, :] for jc in range(2)]
                    else:
                        yps = [ltpool.tile([128, 512], fp32, tag="lt",
                                           name=f"yp_{b}_{jc}")[:]
                               for jc in range(2)]
                    for jc in range(2):
                        nc.tensor.matmul(
                            yps[jc], atT[k][:],
                            wo_sb[:, jc * 512:(jc + 1) * 512],
                            start=True, stop=True)
                    nc.scalar.copy(yo[:, 0:512], yps[0])
                    nc.vector.tensor_copy(yo[:, 512:1024], yps[1])
                else:
                    for jc in range(2):
                        yp = pspool.tile([128, 512], fp32, tag="ps",
                                         name=f"yp_{b}_{jc}")
                        nc.tensor.matmul(
                            yp[:], atT[k][:],
                            wo_sb[:, jc * 512:(jc + 1) * 512],
                            start=True, stop=True)
                        # split the two copies across ACT/DVE for balance
                        if jc == 0:
                            nc.scalar.copy(yo[:, 0:512], yp[:])
                        else:
                            nc.vector.tensor_copy(
                                yo[:, jc * 512:(jc + 1) * 512], yp[:])
                nc.sync.dma_start(y[b * 128:(b + 1) * 128, :], yo[:])

            # (floor_offset, pe_cost_ns, thunk): scales all run first (they
            # are acc's only readers, so the next wave's PV start unblocks
            # early); transposes and output blocks then trickle so the
            # finalize never oversubscribes an iteration's PE slack
            thunks = [(1, 10, "sc", "dve", recip)]
            if last:
                thunks.append((2, 10, "sc", "dve", lambda: scale_ks(0, 2)))
                thunks.append((3, 10, "sc", "act", lambda: scale_act(2)))
                thunks.append((4, 10, "sc", "act", lambda: scale_act(3)))
            else:
                thunks.append((2, 10, "sc", "dve", lambda: scale_ks(0, 4)))
            for k in range(4):
                if last:
                    # tail: engines are draining — emit as soon as inputs
                    # allow so the final blocks pipeline deep
                    thunks.append((3 + 4 * k, 110, "fin", "dve",
                                   lambda k=k: transp(k)))
                    thunks.append((4 + 4 * k, 430, "fin", None,
                                   lambda k=k: yblock(k)))
                else:
                    thunks.append((5 + 6 * k, 110, "fin", "dve",
                                   lambda k=k: transp(k)))
                    thunks.append((7 + 6 * k, 430, "fin", None,
                                   lambda k=k: yblock(k)))
            return thunks

        def emit_iter(w, tb, accs):
            gi = state["gi"]
            lag = {0: LAG_W0, 1: LAG_W1, NWV - 1: LAG_LAST}.get(w, LAG)
            if w >= 2 and w != NWV - 1:
                # smooth the wave-boundary acc handoff (PVlast(w-1) -> recip
                # -> scale -> PV0(w), accpool bufs=1): give the first PVs of
                # the wave extra lag so the serial chain hides under lt/proj
                # work instead of stalling the PE
                lag = max(LAG, 6 - tb)
            lt = lt_holder.pop("lt")
            dve_tbs = DVE_TBS_W0 if w == 0 else DVE_TBS
            # exp in per-head halves: the lt slot's h0 half frees one
            # exp-half earlier, cutting the lt->exp->slot-free round trip
            # below the PE iteration time (the slot ping-pong is only 2 deep;
            # PSUM can't fit 3) — subtile deps let lt(i+2)'s h0 matmul start
            # as soon as exp(i)'s h0 half completes
            if tb in dve_tbs:
                # Schraudolph exp on DVE: affine into int16, bitcast bf16
                pt = ptpool.tile([128, 1024], i16, tag="pt",
                                 name=f"pt_{w}_{tb}")
                for hh in range(2):
                    nc.vector.tensor_scalar(
                        pt[:, hh * 512:(hh + 1) * 512],
                        lt[hh][:], SCH_A, SCH_B,
                        ALU.mult, ALU.add)
            else:
                pt = ptpool.tile([128, 1024], bf16, tag="pt",
                                 name=f"pt_{w}_{tb}")
                for hh in range(2):
                    nc.scalar.activation(pt[:, hh * 512:(hh + 1) * 512],
                                         lt[hh][:],
                                         AF.Exp, scale=0.125)
            if tb + 1 < NT:
                lt_holder["lt"] = emit_lt(w, tb + 1)
            elif w + 1 < NWV:
                lt_holder["lt"] = emit_lt(w + 1, 0)
            live_pv[w] += 1
            cur_eng = "dve" if tb in dve_tbs else "act"
            pending.append([gi + lag, 644 if (w == 0 and V_TRACK) else 217,
                            "pv", None, w, pv_thunk(w, tb, pt, accs)])
            # Pop READY items anywhere in the list (a far-future finalize
            # floor must not head-block the PV stream), but cap the popped
            # PE-ns per iteration, and defer evacuation thunks whose engine
            # just received this iteration's exp (a copy queued between exp
            # halves delays the lt slot free and stalls the PE).
            budget = PE_BUDGET
            i = 0
            while i < len(pending):
                floor, cost, kind, eng, wv, t = pending[i]
                ok = floor <= gi and (cost <= budget or floor <= gi - 12)
                if ok and eng == cur_eng and floor > gi - 6:
                    ok = False
                if ok and kind == "pv":
                    ok = live_sc[wv - 1] == 0
                elif ok:
                    ok = live_pv[wv] == 0
                if ok:
                    pending.pop(i)
                    t()
                    budget -= cost
                    if kind == "pv":
                        live_pv[wv] -= 1
                    elif kind == "sc":
                        live_sc[wv] -= 1
                else:
                    i += 1
            # projection deadlines are HARD (logits read qT/kT at fixed
            # iterations): pop regardless of remaining budget
            pops = 0
            while proj_q and proj_q[0][0] <= gi and pops < 2:
                proj_q.popleft()[2]()
                pops += 1
            state["gi"] = gi + 1

        # ---- main loop ----
        lt_holder["lt"] = emit_lt(0, 0)
        for w in range(NWV):
            accs = []
            for tb in range(NT):
                emit_iter(w, tb, accs)
            for off, cost, kind, eng, t in finalize_thunks(w, accs):
                if kind == "sc":
                    live_sc[w] += 1
                pending.append([state["gi"] + LAG - 2 + off, cost, kind,
                                eng, w, t])
        while proj_q:
            proj_q.popleft()[2]()
        # final drain: keep list order within a wave; gates are satisfied
        # by construction (pv entries precede sc precede fin per wave)
        for e in pending:
            e[5]()

    _split_multi_waits(nc, mybir)
    nc.finalize()
    return nc


def _get_nc():
    if "nc" not in _NC_CACHE:
        _NC_CACHE["nc"] = _build_nc()
    return _NC_CACHE["nc"]


def _relay(wT):
    """[1024 d, 128 m] -> [p, c*m] with wT[c*128+p, m] at [p, c, m]: every
    DMA descriptor becomes a contiguous 2KB run."""
    return np.ascontiguousarray(
        wT.reshape(8, 128, HD).transpose(1, 0, 2).reshape(128, 8 * HD))


def _in_maps(x, Wq, Wk, Wv, Wo):
    import ml_dtypes
    bf16 = ml_dtypes.bfloat16
    xT = np.ascontiguousarray(np.asarray(x, np.float32).T).astype(bf16)
    ident = np.eye(128, dtype=np.float32).astype(bf16)
    maps = []
    for c in range(NCORES):
        sl = slice(HD * c, HD * (c + 1))
        maps.append(dict(
            xT=xT,
            wqT=_relay(np.asarray(Wq)[sl, :].T.astype(bf16)),
            wkT=_relay(np.asarray(Wk)[sl, :].T.astype(bf16)),
            wvT=_relay(np.asarray(Wv)[sl, :].T.astype(bf16)),
            woT=np.ascontiguousarray(np.asarray(Wo)[:, sl].T).astype(bf16),
            ident=ident,
        ))
    return maps


def kernel(x, Wq, Wk, Wv, Wo):
    from concourse.bass_utils import run_bass_kernel_spmd

    x = np.asarray(x, dtype=np.float32)
    nc = _get_nc()
    res = run_bass_kernel_spmd(nc, _in_maps(x, Wq, Wk, Wv, Wo),
                               list(range(NCORES)))
    out = np.zeros((S, D), np.float32)
    for rr in res.results:
        out += np.asarray(rr["y"], dtype=np.float32)
    return out



# revision 55
# speedup vs baseline: 1.3136x; 1.0097x over previous
"""Multi-head attention (S=4096, D=1024, H=16) on 8 trn2 NeuronCores.

Sharding: 2 heads per core (tensor-parallel on Q/K/V column splits and
dense row split). Each core computes a partial [S, D] output; host sums
the 8 partials (the unshard step for row-parallel TP).

Per-core design (bf16 operands, fp32 PSUM accumulate):
  The ACT engine's exp over the S*S*2 logits (256 x [128,1024] tiles,
  ~1.04us each) is the hard floor (~266us busy); everything else is
  scheduled to hide inside it (this build: ~292us total = 10.8us
  startup + 266us exp + ~2.5us exp stalls + ~12.4us finalize tail).
  - logits: lt[128t, 2h*512s] = k-block^T q-window per head (2 matmuls,
    512 rows each).
  - PV streams v, not P: stationary = pt s-block [128t,128s], moving =
    v_aug[128t,65] -> acc[128s,65] accumulated over 32 t-blocks (65
    rows/matmul instead of 512); column 64 of v_aug is ones so the
    softmax denominator lands in acc[...,64] already laid out per
    s-partition. One accumulation group per PSUM 2KB zero-region: only
    (tb0,k0) starts and (tb31,k3) stops each head's bank.
  - normalization: reciprocal of the denominator, then per-partition
    tensor_scalar_mul fused into the acc->SBUF copy.
  - v-projection emits v directly in [s-part, dk] layout (stationary =
    x-block, moving = Wv chunk): no PE transposes of V.
  - output projection: per s-block, transpose normalized attn
    ([128s,64]->[64dk,128s] via PE, 1 cyc/row in bf16), then
    yp[128s,512d] = attnT^T wo; y is bf16, host sums partials in fp32.
  PSUM (8 banks): lt 2x2 (ping-pong), acc 2 (padded [128,2,4,128],
  single-buffered across waves), 2-slot scratch ring shared by proj
  psum / transposes / yp; the last wave's yp also rotates through the
  then-dead lt+acc banks so the tail pipelines 3-deep.
  Schedule: one exp per iteration paces everything. Between consecutive
  lt emissions the PE tolerates only ~1.2us of other work, so PV /
  finalize thunks pop from a floor-gated list under a PE-ns budget;
  projection deadlines are hard (logits read qT/kT at fixed iterations)
  and ordering gates enforce pv(w) < scales(w) < pv(w+1) (acc handoff).
  GPSIMD cannot touch PSUM, so all PSUM evacuation is DVE/ACT.
  Startup: matmul cost is priced at decode time ~36 instructions ahead
  of execution, so 8 big + 36 tiny memset-fed warm-up matmuls both ramp
  the PE p-state and flush the decode queue before the first real
  projections; DMA order puts wk/wq/x-piece-0 first (HWDGE serializes
  at ~625ns per dma_start) and weights are host-re-laid so every DMA
  descriptor is a contiguous 2KB run.
"""

import numpy as np
from collections import deque
from contextlib import ExitStack

S = 4096
D = 1024
NCORES = 8
HD = 128  # head-dim span per core (2 heads x 64)
DK = 64
NT = S // 128   # 32 t-blocks
NWV = 8         # s-waves of 512
LAG_W0 = 18   # wave-0 PV lag: pushes PV+v work past the k/q-proj burst
LAG = 2       # steady-state PV lag (lt(i+1) is emitted before the
              # pops, so a briefly-waiting PV never blocks logits)
LAG_LAST = 2  # final wave drains tight to shorten the tail
PE_BUDGET = 600   # max popped PE-ns per iteration (guards the exp cadence)
V_TRACK = True    # v-blocks emitted from inside wave-0 PV pops (+4 lookahead)
Q_MID = False     # q pieces at wave end (mid-wave collides with finalize)
LAG_W1 = 16       # wave-1 PV lag (wave-0 backlog still draining)

# Schraudolph exp-on-DVE: i16 = trunc/round(A*(0.125*lt) + B); bitcast bf16
# gives e^z * (1 + O(3%)).  A = 128*log2(e) folded with the 0.125 logit
# scale; B centers the linear-in-mantissa interpolation error (calibrated
# against the real convert rounding via test.py sweep).
SCH_A = float(128.0 / np.log(2.0) * 0.125)
SCH_B = 16256.0 - 5.125
# t-blocks whose exp runs on DVE (Schraudolph) instead of ACT, per wave.
# 12/32 offload in an A,A,D-ish weave: ACT does 20 exps/wave (~21us), DVE
# ~14us + evacuations; both below the PE wave time (~27.8us), making PE the
# pacing engine.  tb 0-3 stay on ACT so the wave-boundary recip/scale (the
# serial acc handoff, accpool bufs=1) isn't queued behind a 1.2us DVE exp.
DVE_TBS = {1, 3, 6, 8, 11, 13, 16, 18, 21, 23, 26, 28, 31}
DVE_TBS_W0 = {16, 19, 22, 25, 28, 31}  # wave 0: DVE busy with v/proj backlog

YO_ACT = True   # mid-wave yo copy jc0 on ACT (False -> both halves on DVE)
LAG_RAMP = 8    # wave-boundary smoothing: lag = max(LAG, LAG_RAMP - tb)

import os as _os
if _os.environ.get("KCFG_YO_ACT"):
    YO_ACT = _os.environ["KCFG_YO_ACT"] == "1"
for _k in ("PE_BUDGET", "LAG", "LAG_W0", "LAG_W1", "LAG_LAST", "LAG_RAMP"):
    if _os.environ.get("KCFG_" + _k):
        globals()[_k] = int(_os.environ["KCFG_" + _k])
if _os.environ.get("KCFG_DVE_TBS"):
    DVE_TBS = set(int(x) for x in _os.environ["KCFG_DVE_TBS"].split(","))
if _os.environ.get("KCFG_DVE_TBS_W0"):
    DVE_TBS_W0 = set(int(x) for x in _os.environ["KCFG_DVE_TBS_W0"].split(","))
if _os.environ.get("KCFG_SCH_B"):
    SCH_B = float(_os.environ["KCFG_SCH_B"])

_NC_CACHE = {}


def _split_multi_waits(nc, mybir):
    """This walrus build encodes at most ~2 sync commands per instruction
    (1 for matmul/drain). Keep <=1 wait on every compute/DMA instruction and
    move the rest into standalone dual-condition EventSemaphore instructions
    inserted immediately before it on the same engine (same wait point, so
    semantics are unchanged)."""
    n = 0
    used = set()
    for b in nc.m.functions[0].blocks:
        for inst in b.instructions:
            si = inst.sync_info
            if si:
                for w in (si.on_wait or []):
                    used.add(w.id)
                for u in (si.on_update or []):
                    used.add(u.id)
    free_ids = [i for i in range(max(used) + 1, max(used) + 32)]
    sems = {}

    def eng_sem(eng):
        if eng not in sems:
            sems[eng] = (free_ids.pop(0), f"wsplit_{len(sems)}")
        return sems[eng]

    for b in nc.m.functions[0].blocks:
        il = b.instructions
        new = []
        for inst in il:
            si = inst.sync_info
            waits = list(si.on_wait) if si and si.on_wait else []
            upds = list(si.on_update) if si and si.on_update else []
            if type(inst).__name__ == "InstEventSemaphore":
                new.append(inst)
                continue
            if len(waits) > 1:
                excess, keep = waits[:-1], waits[-1:]
                for i in range(0, len(excess), 2):
                    sid, sname = eng_sem(inst.engine)
                    ev = mybir.InstEventSemaphore(
                        name=f"{inst.name}_ws{i}", engine=inst.engine,
                        ins=[], outs=[],
                        sync_info=mybir.SyncInfo(
                            on_wait=excess[i:i + 2],
                            on_update=[mybir.SyncUpdate(
                                sync_type="semaphore", id=sid,
                                ant_name=sname, update_mode="sem-inc",
                                update_value=1, update_reg=None)]))
                    new.append(ev)
                    n += 1
                inst.sync_info = mybir.SyncInfo(on_wait=keep, on_update=upds)
            new.append(inst)
        il[:] = new
    return n


def _build_nc():
    import concourse.bass as bass
    import concourse.tile as tile
    import concourse.mybir as mybir

    fp32 = mybir.dt.float32
    bf16 = mybir.dt.bfloat16
    i16 = mybir.dt.int16
    AF = mybir.ActivationFunctionType
    ALU = mybir.AluOpType

    nc = bass.Bass()
    xT = nc.dram_tensor("xT", [D, S], bf16, kind="ExternalInput")
    # weights pre-laid host-side as [p, c, m] so each DMA descriptor is a
    # contiguous 2KB run (256B descriptors pay a 2x latency penalty)
    wqT = nc.dram_tensor("wqT", [128, 8 * HD], bf16, kind="ExternalInput")
    wkT = nc.dram_tensor("wkT", [128, 8 * HD], bf16, kind="ExternalInput")
    wvT = nc.dram_tensor("wvT", [128, 8 * HD], bf16, kind="ExternalInput")
    woT = nc.dram_tensor("woT", [HD, D], bf16, kind="ExternalInput")
    ident = nc.dram_tensor("ident", [128, 128], bf16, kind="ExternalInput")
    y = nc.dram_tensor("y", [S, D], bf16, kind="ExternalOutput")

    with tile.TileContext(nc) as tc, ExitStack() as ctx, \
         nc.allow_low_precision(reason="bf16 operands within rel-err budget"):
        sb = ctx.enter_context(tc.tile_pool(name="sb", bufs=1))
        qT_sb = sb.tile([128, S], bf16, tag="qT")
        kT_sb = sb.tile([128, S], bf16, tag="kT")
        # v_aug[:, tb, h, 0:64] = v block for head h; [..., 64] = ones
        v_aug = sb.tile([128, NT, 2, DK + 1], bf16, tag="vaug")
        wq_sb = sb.tile([128, 8, HD], bf16, tag="wq")
        wk_sb = sb.tile([128, 8, HD], bf16, tag="wk")
        wv_sb = sb.tile([128, 8, HD], bf16, tag="wv")
        wo_sb = sb.tile([HD, D], bf16, tag="wo")
        id_sb = sb.tile([128, 128], bf16, tag="id")
        warm = sb.tile([1, DK], fp32, tag="warm")
        wsrc = sb.tile([128, 512], bf16, tag="wsrc")

        xpool = ctx.enter_context(tc.tile_pool(name="xpool", bufs=8))
        ptpool = ctx.enter_context(tc.tile_pool(name="ptpool", bufs=LAG_W0 + 14))
        attnpool = ctx.enter_context(tc.tile_pool(name="attnpool", bufs=2))
        atTpool = ctx.enter_context(tc.tile_pool(name="atTpool", bufs=4))
        yopool = ctx.enter_context(tc.tile_pool(name="yopool", bufs=8))
        dpool = ctx.enter_context(tc.tile_pool(name="dpool", bufs=2))
        ltpool = ctx.enter_context(
            tc.tile_pool(name="ltpool", bufs=4, space="PSUM"))
        accpool = ctx.enter_context(
            tc.tile_pool(name="accpool", bufs=1, space="PSUM"))
        pspool = ctx.enter_context(
            tc.tile_pool(name="pspool", bufs=2, space="PSUM"))

        # ---- startup DMAs: critical path (wk, wq, x piece 0) first ----
        nc.sync.dma_start(wk_sb[:], wkT.rearrange("p (c m) -> p c m", c=8))
        nc.sync.dma_start(wq_sb[:], wqT.rearrange("p (c m) -> p c m", c=8))
        xTr = xT.rearrange("(c p) s -> p c s", p=128)
        xqs = {}

        def load_xq(p):
            xq = xpool.tile([128, 8, 512], bf16, tag="xq", name=f"xq_{p}")
            for c in range(0, 8, 2):
                nc.sync.dma_start(xq[:, c:c + 2, :],
                                  xTr[:, c:c + 2, p * 512:(p + 1) * 512])
            xqs[p] = xq

        load_xq(0)
        nc.sync.dma_start(wv_sb[:], wvT.rearrange("p (c m) -> p c m", c=8))
        nc.sync.dma_start(wo_sb[:], woT[:])
        nc.sync.dma_start(id_sb[:], ident[:])
        # Warm-up fed by an on-chip memset (no DMA dependency). Matmul cost
        # is priced at DECODE time, which runs ~36 instructions (exec-queue
        # depth) ahead of execution: to get the real projection matmuls
        # priced at the full-speed p-state, the PE must (a) be continuously
        # busy >3us and (b) have >=36 instructions in flight before them.
        # 8 big warms ramp the clock, 36 tiny ones flush the decode queue.
        # wsrc memset first: the warm matmuls wait on it.
        nc.vector.memset(wsrc[:], 0.5)
        nc.vector.memset(v_aug[:, :, :, DK], 1.0)
        nc.scalar.activation(warm[:], wsrc[0:1, 0:DK], AF.Exp, scale=0.125)
        warm_slots = [
            pspool.tile([128, 512], fp32, tag="ps", name=f"warmps_{i}")
            for i in range(2)]
        warm_lt = [
            ltpool.tile([128, 512], fp32, tag="lt", name=f"warmlt_{i}")
            for i in range(4)]
        for i in range(8):
            wp = (warm_slots[i % 2][:, 0:512] if i % 4 < 2
                  else warm_lt[i % 4][:])
            nc.tensor.matmul(wp, wsrc[:, 0:128], wsrc[:, 0:512],
                             start=True, stop=True)
        for i in range(36):
            wp = (warm_slots[i % 2][:, 0:32] if i % 4 < 2
                  else warm_lt[i % 4][:, 0:32])
            nc.tensor.matmul(wp, wsrc[:, 0:128], wsrc[:, 0:32],
                             start=True, stop=True)
        # remaining x pieces; all 8 stay resident (q-proj of piece p runs
        # ~4 waves after its k-proj, so slots can't rotate)
        for p in range(1, 8):
            load_xq(p)

        # ---- projection pieces (as resumable quarter thunks) ----
        def qk_proj_thunks(p, w_sb, dst):
            """Four thunks of 2 chunks each for dst[:, p*512:(p+1)*512]."""
            hold = {}

            def quarter(i):
                def run():
                    if i == 0:
                        hold["ps"] = pspool.tile(
                            [128, 512], fp32, tag="ps",
                            name=f"pqk_{dst.tensor.name}_{p}")
                    ps = hold["ps"]
                    for c in range(2 * i, 2 * i + 2):
                        nc.tensor.matmul(ps[:], w_sb[:, c, :],
                                         xqs[p][:, c, :], start=(c == 0),
                                         stop=(c == 7))
                    if i == 3:
                        nc.vector.tensor_copy(
                            dst[:, p * 512:(p + 1) * 512], hold.pop("ps")[:])
                return run
            return [quarter(i) for i in range(4)]

        def v_block_thunk(tb):
            def run():
                p, i = tb // 4, tb % 4
                vps = pspool.tile([128, 128], fp32, tag="ps",
                                  name=f"vps_{tb}")
                for c in range(8):
                    nc.tensor.matmul(
                        vps[:], xqs[p][:, c, i * 128:(i + 1) * 128],
                        wv_sb[:, c, :], start=(c == 0), stop=(c == 7))
                nc.vector.tensor_copy(
                    v_aug[:, tb, :, 0:DK],
                    vps[:].rearrange("p (h d) -> p h d", h=2))
            return run

        # ---- startup projections: piece 0 of q and k, interleaved per
        # x-chunk so matmuls start as soon as each chunk DMA lands; the
        # two PSUM->SBUF copies run on different engines in parallel ----
        pq0 = pspool.tile([128, 512], fp32, tag="ps", name="pq0")
        pk0 = pspool.tile([128, 512], fp32, tag="ps", name="pk0")
        xq0 = xqs[0]
        for c in range(8):
            nc.tensor.matmul(pq0[:], wq_sb[:, c, :], xq0[:, c, :],
                             start=(c == 0), stop=(c == 7))
            nc.tensor.matmul(pk0[:], wk_sb[:, c, :], xq0[:, c, :],
                             start=(c == 0), stop=(c == 7))
        nc.scalar.copy(qT_sb[:, 0:512], pq0[:])
        # lt(0,0) needs only k block 0: give it its own small copy
        nc.vector.tensor_copy(kT_sb[:, 0:128], pk0[:, 0:128])
        nc.vector.tensor_copy(kT_sb[:, 128:512], pk0[:, 128:512])

        # deadline-ordered weave of the remaining projection work; each
        # entry is [deadline, pe_cost_ns, thunk]
        proj_q = deque()
        for p in range(1, 8):
            for j, t in enumerate(qk_proj_thunks(p, wk_sb, kT_sb)):
                proj_q.append([4 * p - 5 + j, 427, t])
        # v blocks 0..3 by deadline; the rest are emitted from inside the
        # wave-0 PV pops with a +4 lookahead, auto-tracking the drain rate
        v_done = set()

        def ensure_v(tb):
            if tb < NT and tb not in v_done:
                v_done.add(tb)
                v_block_thunk(tb)()

        n_pre_v = 4 if V_TRACK else NT
        for tb in range(n_pre_v):
            proj_q.append([LAG_W0 - 6 + tb, 427, lambda tb=tb: ensure_v(tb)])
        # q piece w must be fully copied before lt(w, 0) is emitted at
        # gi 32w-1: the last quarter lands at base+6 <= 32w-3
        for j, t in enumerate(qk_proj_thunks(1, wq_sb, qT_sb)):
            proj_q.append([(16 if Q_MID else 22) + 2 * j, 427, t])
        for w in range(2, NWV):
            for j, t in enumerate(qk_proj_thunks(w, wq_sb, qT_sb)):
                base = 32 * (w - 1) + (10 if Q_MID else 22)
                proj_q.append([base + 2 * j, 427, t])
        proj_q = deque(sorted(proj_q, key=lambda e: e[0]))

        # ---- attention machinery ----
        # pending: [floor, cost, kind, wave, thunk]; kind "pv" | "sc" | "fin".
        # Ordering gates (enforced at pop time, not just via floors):
        #   fin/sc(w) pop only after all pv(w) popped;
        #   pv(w+1) pops only after all sc(w) popped (acc slot handoff).
        pending = []
        live_pv = {w: 0 for w in range(NWV)}
        live_sc = {w: 0 for w in range(-1, NWV)}
        live_sc[-1] = 0
        state = {"gi": 0}
        lt_holder = {}

        def emit_lt(w, tb):
            # per-head lt tiles (1 PSUM bank each): each head's
            # lt -> exp -> slot-free chain ping-pongs independently across
            # 2 of the 4 slots, halving the latency the cadence must absorb
            s0 = w * 512
            lts = []
            for h in range(2):
                lt_h = ltpool.tile([128, 512], fp32, tag="lt",
                                   name=f"lt_{w}_{tb}_{h}")
                nc.tensor.matmul(
                    lt_h[:],
                    kT_sb[DK * h:DK * (h + 1), tb * 128:(tb + 1) * 128],
                    qT_sb[DK * h:DK * (h + 1), s0:s0 + 512],
                    start=True, stop=True,
                    tile_position=(DK * h, 0),
                )
                lts.append(lt_h)
            return lts

        def pv_thunk(w, tb, pt, accs):
            def run():
                if w == 0 and V_TRACK:
                    ensure_v(tb)
                    ensure_v(tb + 4)
                if tb == 0:
                    accs.append(accpool.tile([128, 2, 4, 128], fp32,
                                             tag="acc", name=f"acc_{w}"))
                acc = accs[0]
                p_all = (pt[:].bitcast(bf16) if pt.tensor.dtype == i16
                         else pt[:])
                for h in range(2):
                    for k in range(4):
                        # one accumulation group per PSUM zero-region (the
                        # 2KB bank holding all 4 k-slices of head h): start
                        # marks the whole bank pending-zero, so only the
                        # first slice may start and only the last may stop;
                        # the other tb==0 writes zero-on-first-touch.
                        nc.tensor.matmul(
                            acc[:, h, k, 0:DK + 1],
                            p_all[:, h * 512 + k * 128:h * 512 + (k + 1) * 128],
                            v_aug[:, tb, h, :],
                            start=(tb == 0 and k == 0),
                            stop=(tb == NT - 1 and k == 3),
                            skip_group_check=(k != 0),
                        )
            return run

        def finalize_thunks(w, accs):
            rden = dpool.tile([128, 2, 4], fp32, tag="rden", name=f"rden_{w}")
            attn_all = attnpool.tile([128, 2, 4, DK], bf16, tag="attn",
                                     name=f"attn_{w}")
            atT = {}

            def recip():
                nc.vector.reciprocal(rden[:], accs[0][:, :, :, DK])

            def scale_ks(k0, k1):
                # one DVE op normalizes all (h, k0:k1) blocks: rden broadcast
                # along dk via a stride-0 free dim
                nc.vector.tensor_tensor(
                    attn_all[:, :, k0:k1, :], accs[0][:, :, k0:k1, 0:DK],
                    rden[:, :, k0:k1, None].to_broadcast([128, 2, k1 - k0, DK]),
                    ALU.mult)

            def scale_act(k):
                # last wave: late blocks on the otherwise-idle ACT so their
                # transposes unblock without waiting out DVE's queue
                for h in range(2):
                    nc.scalar.activation(attn_all[:, h, k, :],
                                         accs[0][:, h, k, 0:DK],
                                         AF.Copy, scale=rden[:, h, k:k + 1])

            last = w == NWV - 1

            def transp(k):
                # both heads transpose into one [128,128] psum tile (disjoint
                # partition ranges -> no zero-region conflict), one copy out
                atT[k] = atTpool.tile([128, 128], bf16, tag="atT",
                                      name=f"atT_{w}_{k}")
                tps = pspool.tile([128, 128], bf16, tag="ps",
                                  name=f"tps_{w}_{k}")
                for h in range(2):
                    nc.tensor.transpose(tps[DK * h:DK * (h + 1), :],
                                        attn_all[:, h, k, :], id_sb[:])
                nc.vector.tensor_copy(atT[k][:], tps[:])

            def yblock(k):
                # one combined [128,1024] output + ONE y DMA per s-block:
                # HWDGE serializes issues at ~625ns, so halving the DMA
                # count shortens the end-of-kernel trickle
                b = w * 4 + k
                yo = yopool.tile([128, 1024], bf16, tag="yo",
                                 name=f"yo_{b}")
                if last:
                    # the lt slots are dead after the final exp and the acc
                    # banks after the scales: draw per-half yp banks from
                    # them so the tail pipelines 4+ deep, and split the
                    # copies across the idle ACT and DVE
                    if k == 3:
                        # acc banks free only after every scale has read
                        # them — give them to the LAST block
                        ypt = accpool.tile([128, 2, 512], fp32, tag="acc",
                                           name=f"yp_{b}")
                        yps = [ypt[:, jc, :] for jc in range(2)]
                    else:
                        yps = [ltpool.tile([128, 512], fp32, tag="lt",
                                           name=f"yp_{b}_{jc}")[:]
                               for jc in range(2)]
                    for jc in range(2):
                        nc.tensor.matmul(
                            yps[jc], atT[k][:],
                            wo_sb[:, jc * 512:(jc + 1) * 512],
                            start=True, stop=True)
                    nc.scalar.copy(yo[:, 0:512], yps[0])
                    nc.vector.tensor_copy(yo[:, 512:1024], yps[1])
                else:
                    for jc in range(2):
                        yp = pspool.tile([128, 512], fp32, tag="ps",
                                         name=f"yp_{b}_{jc}")
                        nc.tensor.matmul(
                            yp[:], atT[k][:],
                            wo_sb[:, jc * 512:(jc + 1) * 512],
                            start=True, stop=True)
                        if jc == 0 and YO_ACT:
                            nc.scalar.copy(yo[:, 0:512], yp[:])
                        else:
                            nc.vector.tensor_copy(
                                yo[:, jc * 512:(jc + 1) * 512], yp[:])
                nc.sync.dma_start(y[b * 128:(b + 1) * 128, :], yo[:])

            # (floor_offset, pe_cost_ns, thunk): scales all run first (they
            # are acc's only readers, so the next wave's PV start unblocks
            # early); transposes and output blocks then trickle so the
            # finalize never oversubscribes an iteration's PE slack
            if last:
                thunks = [(1, 10, "sc", "dve", recip)]
                thunks.append((2, 10, "sc", "dve", lambda: scale_ks(0, 2)))
                thunks.append((3, 10, "sc", "act", lambda: scale_act(2)))
                thunks.append((4, 10, "sc", "act", lambda: scale_act(3)))
            else:
                thunks = [(1, 10, "sc", "dve", recip)]
                thunks.append((2, 10, "sc", "dve", lambda: scale_ks(0, 4)))
            for k in range(4):
                if last:
                    # tail: engines are draining — emit as soon as inputs
                    # allow so the final blocks pipeline deep
                    thunks.append((3 + 4 * k, 110, "fin", "dve",
                                   lambda k=k: transp(k)))
                    thunks.append((4 + 4 * k, 430, "fin", None,
                                   lambda k=k: yblock(k)))
                else:
                    thunks.append((5 + 6 * k, 110, "fin", "dve",
                                   lambda k=k: transp(k)))
                    thunks.append((7 + 6 * k, 430, "fin", "dve",
                                   lambda k=k: yblock(k)))
            return thunks

        def emit_iter(w, tb, accs):
            gi = state["gi"]
            lag = {0: LAG_W0, 1: LAG_W1, NWV - 1: LAG_LAST}.get(w, LAG)
            if w >= 2 and w != NWV - 1:
                # smooth the wave-boundary acc handoff (PVlast(w-1) -> recip
                # -> scale -> PV0(w), accpool bufs=1): give the first PVs of
                # the wave extra lag so the serial chain hides under lt/proj
                # work instead of stalling the PE
                lag = max(LAG, LAG_RAMP - tb)
            lt = lt_holder.pop("lt")
            dve_tbs = DVE_TBS_W0 if w == 0 else DVE_TBS
            # exp in per-head halves: the lt slot's h0 half frees one
            # exp-half earlier, cutting the lt->exp->slot-free round trip
            # below the PE iteration time (the slot ping-pong is only 2 deep;
            # PSUM can't fit 3) — subtile deps let lt(i+2)'s h0 matmul start
            # as soon as exp(i)'s h0 half completes
            if tb in dve_tbs:
                # Schraudolph exp on DVE: affine into int16, bitcast bf16
                pt = ptpool.tile([128, 1024], i16, tag="pt",
                                 name=f"pt_{w}_{tb}")
                for hh in range(2):
                    nc.vector.tensor_scalar(
                        pt[:, hh * 512:(hh + 1) * 512],
                        lt[hh][:], SCH_A, SCH_B,
                        ALU.mult, ALU.add)
            else:
                pt = ptpool.tile([128, 1024], bf16, tag="pt",
                                 name=f"pt_{w}_{tb}")
                for hh in range(2):
                    nc.scalar.activation(pt[:, hh * 512:(hh + 1) * 512],
                                         lt[hh][:],
                                         AF.Exp, scale=0.125)
            if tb + 1 < NT:
                lt_holder["lt"] = emit_lt(w, tb + 1)
            elif w + 1 < NWV:
                lt_holder["lt"] = emit_lt(w + 1, 0)
            live_pv[w] += 1
            cur_eng = "dve" if tb in dve_tbs else "act"
            nxt_eng = "dve" if (tb + 1) in dve_tbs else "act"
            pending.append([gi + lag, 644 if (w == 0 and V_TRACK) else 217,
                            "pv", None, w, pv_thunk(w, tb, pt, accs)])
            # Pop READY items anywhere in the list (a far-future finalize
            # floor must not head-block the PV stream), but cap the popped
            # PE-ns per iteration, and defer evacuation thunks whose engine
            # just received this iteration's exp (a copy queued between exp
            # halves delays the lt slot free and stalls the PE).
            budget = PE_BUDGET
            i = 0
            while i < len(pending):
                floor, cost, kind, eng, wv, t = pending[i]
                ok = floor <= gi and (cost <= budget or floor <= gi - 12)
                if ok and eng == cur_eng and floor > gi - 6:
                    ok = False
                if ok and kind == "pv":
                    ok = live_sc[wv - 1] == 0
                elif ok:
                    ok = live_pv[wv] == 0
                if ok:
                    pending.pop(i)
                    t()
                    budget -= cost
                    if kind == "pv":
                        live_pv[wv] -= 1
                    elif kind == "sc":
                        live_sc[wv] -= 1
                else:
                    i += 1
            # projection deadlines are HARD (logits read qT/kT at fixed
            # iterations): pop regardless of remaining budget
            pops = 0
            while proj_q and proj_q[0][0] <= gi and pops < 2:
                proj_q.popleft()[2]()
                pops += 1
            state["gi"] = gi + 1

        # ---- main loop ----
        lt_holder["lt"] = emit_lt(0, 0)
        for w in range(NWV):
            accs = []
            for tb in range(NT):
                emit_iter(w, tb, accs)
            for off, cost, kind, eng, t in finalize_thunks(w, accs):
                if kind == "sc":
                    live_sc[w] += 1
                pending.append([state["gi"] + LAG - 2 + off, cost, kind,
                                eng, w, t])
        while proj_q:
            proj_q.popleft()[2]()
        # final drain: keep list order within a wave; gates are satisfied
        # by construction (pv entries precede sc precede fin per wave)
        for e in pending:
            e[5]()

    _split_multi_waits(nc, mybir)
    nc.finalize()
    return nc


def _get_nc():
    if "nc" not in _NC_CACHE:
        _NC_CACHE["nc"] = _build_nc()
    return _NC_CACHE["nc"]


def _relay(wT):
    """[1024 d, 128 m] -> [p, c*m] with wT[c*128+p, m] at [p, c, m]: every
    DMA descriptor becomes a contiguous 2KB run."""
    return np.ascontiguousarray(
        wT.reshape(8, 128, HD).transpose(1, 0, 2).reshape(128, 8 * HD))


def _in_maps(x, Wq, Wk, Wv, Wo):
    import ml_dtypes
    bf16 = ml_dtypes.bfloat16
    xT = np.ascontiguousarray(np.asarray(x, np.float32).T).astype(bf16)
    ident = np.eye(128, dtype=np.float32).astype(bf16)
    maps = []
    for c in range(NCORES):
        sl = slice(HD * c, HD * (c + 1))
        maps.append(dict(
            xT=xT,
            wqT=_relay(np.asarray(Wq)[sl, :].T.astype(bf16)),
            wkT=_relay(np.asarray(Wk)[sl, :].T.astype(bf16)),
            wvT=_relay(np.asarray(Wv)[sl, :].T.astype(bf16)),
            woT=np.ascontiguousarray(np.asarray(Wo)[:, sl].T).astype(bf16),
            ident=ident,
        ))
    return maps


def kernel(x, Wq, Wk, Wv, Wo):
    from concourse.bass_utils import run_bass_kernel_spmd

    x = np.asarray(x, dtype=np.float32)
    nc = _get_nc()
    res = run_bass_kernel_spmd(nc, _in_maps(x, Wq, Wk, Wv, Wo),
                               list(range(NCORES)))
    out = np.zeros((S, D), np.float32)
    for rr in res.results:
        out += np.asarray(rr["y"], dtype=np.float32)
    return out



# revision 62
# speedup vs baseline: 1.3139x; 1.0002x over previous
"""Multi-head attention (S=4096, D=1024, H=16) on 8 trn2 NeuronCores.

Sharding: 2 heads per core (tensor-parallel on Q/K/V column splits and
dense row split). Each core computes a partial [S, D] output; host sums
the 8 partials (the unshard step for row-parallel TP).

Per-core design (bf16 operands, fp32 PSUM accumulate):
  The softmax exp over the S*S*2 logits is SPLIT between ACT (19/32
  t-blocks, native Exp) and DVE (13/32, Schraudolph: one tensor_scalar
  computing i16 = trunc(A*z + B) whose int16 bits, bitcast to bf16, ARE
  e^z to ~3.3%; PV reads the bitcast view, so the approx exp costs ONE
  1x-rate DVE op). That demotes exp from the bottleneck (~266us on ACT
  alone) to a shared load (~216us ACT / ~180us DVE incl. evacuations),
  making the PE the pacing engine (~226us busy, 91%): logits 109us at
  the dk=64 half-array rate + PV 56 + projections 41 + y-proj 14 +
  transposes/warm ~6. This build: ~249us = 10us startup + PE-paced
  steady waves (~8us residual lt-slot stalls) + ~10us finalize tail.
  Error: Schraudolph on 40% of keys -> median rel err 8.9e-3 (vs 5.3e-3
  all-exact), absmax ratio 1.5e-2, within the 2e-2 gate.
  - logits: per-head lt tiles [128t, 512s] (ONE psum bank each, 4-slot
    rotation = per-head 2-deep ping-pong): the slot-free latency chain
    (lt write -> exp -> slot free) spans 2 iterations per head, so the
    ~880ns/iter PE cadence tolerates ~770ns of exp-engine queue jitter;
    exp is emitted per head-half to free each slot as early as possible.
  - PV streams v, not P: stationary = pt s-block [128t,128s], moving =
    v_aug[128t,65] -> acc[128s,65] accumulated over 32 t-blocks (65
    rows/matmul instead of 512); column 64 of v_aug is ones so the
    softmax denominator lands in acc[...,64] already laid out per
    s-partition. One accumulation group per PSUM 2KB zero-region: only
    (tb0,k0) starts and (tb31,k3) stops each head's bank.
  - normalization: reciprocal of the denominator, then ONE broadcast
    tensor_tensor (rden stride-0 along dk) normalizes all 8 (h,k)
    blocks per wave.
  - v-projection emits v directly in [s-part, dk] layout (stationary =
    x-block, moving = Wv chunk): no PE transposes of V.
  - output projection: per s-block, transpose normalized attn (both
    heads into one [128,128] psum tile, disjoint partition quadrants),
    then yp[128s,512d] = attnT^T wo; the two yo copies split ACT/DVE;
    y is bf16, host sums partials in fp32.
  PSUM (8 banks): lt 4x1 (per-head slots), acc 2 (padded [128,2,4,128],
  single-buffered across waves), 2-slot scratch ring shared by proj
  psum / transposes / yp; the last wave's yp rotates through the
  then-dead lt banks (k=3 takes acc, which frees last) so the tail
  pipelines 4-deep.
  Schedule: PE paces; one exp per iteration drains the lt slots. PV /
  finalize thunks pop from a floor-gated list under a PE-ns budget;
  evacuation thunks are engine-tagged and deferred when this
  iteration's exp runs on the same engine (a copy queued between exp
  halves delays the slot free and stalls the PE); finalize floors are
  placed at iterations whose exp neighbors run on the other engine.
  Projection deadlines are hard (logits read qT/kT at fixed iterations)
  and ordering gates enforce pv(w) < scales(w) < pv(w+1) (acc handoff);
  the first PVs of each wave get a lag ramp so the serial acc handoff
  (PVlast -> recip -> scale -> PV0, accpool bufs=1) hides under lt/proj
  work. GPSIMD cannot touch PSUM, so all PSUM evacuation is DVE/ACT.
  Startup: matmul cost is priced at decode time ~36 instructions ahead
  of execution, so 8 big + 36 tiny memset-fed warm-up matmuls both ramp
  the PE p-state and flush the decode queue before the first real
  projections; DMA order puts wk/wq/x-piece-0 first (HWDGE serializes
  at ~625ns per dma_start) and weights are host-re-laid so every DMA
  descriptor is a contiguous 2KB run.
"""

import numpy as np
from collections import deque
from contextlib import ExitStack

S = 4096
D = 1024
NCORES = 8
HD = 128  # head-dim span per core (2 heads x 64)
DK = 64
NT = S // 128   # 32 t-blocks
NWV = 8         # s-waves of 512
LAG_W0 = 18   # wave-0 PV lag: pushes PV+v work past the k/q-proj burst
LAG = 2       # steady-state PV lag (lt(i+1) is emitted before the
              # pops, so a briefly-waiting PV never blocks logits)
LAG_LAST = 2  # final wave drains tight to shorten the tail
PE_BUDGET = 600   # max popped PE-ns per iteration (guards the exp cadence)
V_TRACK = True    # v-blocks emitted from inside wave-0 PV pops (+4 lookahead)
Q_MID = False     # q pieces at wave end (mid-wave collides with finalize)
LAG_W1 = 16       # wave-1 PV lag (wave-0 backlog still draining)

# Schraudolph exp-on-DVE: i16 = trunc/round(A*(0.125*lt) + B); bitcast bf16
# gives e^z * (1 + O(3%)).  A = 128*log2(e) folded with the 0.125 logit
# scale; B centers the linear-in-mantissa interpolation error (calibrated
# against the real convert rounding via test.py sweep).
SCH_A = float(128.0 / np.log(2.0) * 0.125)
SCH_B = 16256.0 - 5.125
# t-blocks whose exp runs on DVE (Schraudolph) instead of ACT, per wave.
# 12/32 offload in an A,A,D-ish weave: ACT does 20 exps/wave (~21us), DVE
# ~14us + evacuations; both below the PE wave time (~27.8us), making PE the
# pacing engine.  tb 0-3 stay on ACT so the wave-boundary recip/scale (the
# serial acc handoff, accpool bufs=1) isn't queued behind a 1.2us DVE exp.
DVE_TBS = {1, 3, 6, 8, 11, 13, 16, 18, 21, 23, 26, 28, 31}
DVE_TBS_W0 = {16, 19, 22, 25, 28, 31}  # wave 0: DVE busy with v/proj backlog

YO_ACT = True   # mid-wave yo copy jc0 on ACT (False -> both halves on DVE)
LAG_RAMP = 8    # wave-boundary smoothing: lag = max(LAG, LAG_RAMP - tb)
RECIP_F = 4     # recip/scale floor (popped at an ACT,ACT-neighbor iter)
TR_F = [4, 9, 14, 24]    # transpose floors (steady waves)
YB_F = [7, 14, 20, 27]   # y-block floors (steady waves)

_NC_CACHE = {}


def _split_multi_waits(nc, mybir):
    """This walrus build encodes at most ~2 sync commands per instruction
    (1 for matmul/drain). Keep <=1 wait on every compute/DMA instruction and
    move the rest into standalone dual-condition EventSemaphore instructions
    inserted immediately before it on the same engine (same wait point, so
    semantics are unchanged)."""
    n = 0
    used = set()
    for b in nc.m.functions[0].blocks:
        for inst in b.instructions:
            si = inst.sync_info
            if si:
                for w in (si.on_wait or []):
                    used.add(w.id)
                for u in (si.on_update or []):
                    used.add(u.id)
    free_ids = [i for i in range(max(used) + 1, max(used) + 32)]
    sems = {}

    def eng_sem(eng):
        if eng not in sems:
            sems[eng] = (free_ids.pop(0), f"wsplit_{len(sems)}")
        return sems[eng]

    for b in nc.m.functions[0].blocks:
        il = b.instructions
        new = []
        for inst in il:
            si = inst.sync_info
            waits = list(si.on_wait) if si and si.on_wait else []
            upds = list(si.on_update) if si and si.on_update else []
            if type(inst).__name__ == "InstEventSemaphore":
                new.append(inst)
                continue
            if len(waits) > 1:
                excess, keep = waits[:-1], waits[-1:]
                for i in range(0, len(excess), 2):
                    sid, sname = eng_sem(inst.engine)
                    ev = mybir.InstEventSemaphore(
                        name=f"{inst.name}_ws{i}", engine=inst.engine,
                        ins=[], outs=[],
                        sync_info=mybir.SyncInfo(
                            on_wait=excess[i:i + 2],
                            on_update=[mybir.SyncUpdate(
                                sync_type="semaphore", id=sid,
                                ant_name=sname, update_mode="sem-inc",
                                update_value=1, update_reg=None)]))
                    new.append(ev)
                    n += 1
                inst.sync_info = mybir.SyncInfo(on_wait=keep, on_update=upds)
            new.append(inst)
        il[:] = new
    return n


def _build_nc():
    import concourse.bass as bass
    import concourse.tile as tile
    import concourse.mybir as mybir

    fp32 = mybir.dt.float32
    bf16 = mybir.dt.bfloat16
    i16 = mybir.dt.int16
    AF = mybir.ActivationFunctionType
    ALU = mybir.AluOpType

    nc = bass.Bass()
    xT = nc.dram_tensor("xT", [D, S], bf16, kind="ExternalInput")
    # weights pre-laid host-side as [p, c, m] so each DMA descriptor is a
    # contiguous 2KB run (256B descriptors pay a 2x latency penalty)
    wqT = nc.dram_tensor("wqT", [128, 8 * HD], bf16, kind="ExternalInput")
    wkT = nc.dram_tensor("wkT", [128, 8 * HD], bf16, kind="ExternalInput")
    wvT = nc.dram_tensor("wvT", [128, 8 * HD], bf16, kind="ExternalInput")
    woT = nc.dram_tensor("woT", [HD, D], bf16, kind="ExternalInput")
    ident = nc.dram_tensor("ident", [128, 128], bf16, kind="ExternalInput")
    y = nc.dram_tensor("y", [S, D], bf16, kind="ExternalOutput")

    with tile.TileContext(nc) as tc, ExitStack() as ctx, \
         nc.allow_low_precision(reason="bf16 operands within rel-err budget"):
        sb = ctx.enter_context(tc.tile_pool(name="sb", bufs=1))
        qT_sb = sb.tile([128, S], bf16, tag="qT")
        kT_sb = sb.tile([128, S], bf16, tag="kT")
        # v_aug[:, tb, h, 0:64] = v block for head h; [..., 64] = ones
        v_aug = sb.tile([128, NT, 2, DK + 1], bf16, tag="vaug")
        wq_sb = sb.tile([128, 8, HD], bf16, tag="wq")
        wk_sb = sb.tile([128, 8, HD], bf16, tag="wk")
        wv_sb = sb.tile([128, 8, HD], bf16, tag="wv")
        wo_sb = sb.tile([HD, D], bf16, tag="wo")
        id_sb = sb.tile([128, 128], bf16, tag="id")
        warm = sb.tile([1, DK], fp32, tag="warm")
        wsrc = sb.tile([128, 512], bf16, tag="wsrc")

        xpool = ctx.enter_context(tc.tile_pool(name="xpool", bufs=8))
        ptpool = ctx.enter_context(tc.tile_pool(name="ptpool", bufs=LAG_W0 + 14))
        attnpool = ctx.enter_context(tc.tile_pool(name="attnpool", bufs=2))
        atTpool = ctx.enter_context(tc.tile_pool(name="atTpool", bufs=4))
        yopool = ctx.enter_context(tc.tile_pool(name="yopool", bufs=8))
        dpool = ctx.enter_context(tc.tile_pool(name="dpool", bufs=2))
        ltpool = ctx.enter_context(
            tc.tile_pool(name="ltpool", bufs=4, space="PSUM"))
        accpool = ctx.enter_context(
            tc.tile_pool(name="accpool", bufs=1, space="PSUM"))
        pspool = ctx.enter_context(
            tc.tile_pool(name="pspool", bufs=2, space="PSUM"))

        # ---- startup DMAs: critical path (wk, wq, x piece 0) first ----
        nc.sync.dma_start(wk_sb[:], wkT.rearrange("p (c m) -> p c m", c=8))
        nc.sync.dma_start(wq_sb[:], wqT.rearrange("p (c m) -> p c m", c=8))
        xTr = xT.rearrange("(c p) s -> p c s", p=128)
        xqs = {}

        def load_xq(p):
            xq = xpool.tile([128, 8, 512], bf16, tag="xq", name=f"xq_{p}")
            for c in range(0, 8, 2):
                nc.sync.dma_start(xq[:, c:c + 2, :],
                                  xTr[:, c:c + 2, p * 512:(p + 1) * 512])
            xqs[p] = xq

        load_xq(0)
        nc.sync.dma_start(wv_sb[:], wvT.rearrange("p (c m) -> p c m", c=8))
        nc.sync.dma_start(wo_sb[:], woT[:])
        nc.sync.dma_start(id_sb[:], ident[:])
        # Warm-up fed by an on-chip memset (no DMA dependency). Matmul cost
        # is priced at DECODE time, which runs ~36 instructions (exec-queue
        # depth) ahead of execution: to get the real projection matmuls
        # priced at the full-speed p-state, the PE must (a) be continuously
        # busy >3us and (b) have >=36 instructions in flight before them.
        # 8 big warms ramp the clock, 36 tiny ones flush the decode queue.
        # wsrc memset first: the warm matmuls wait on it.
        nc.vector.memset(wsrc[:], 0.5)
        nc.vector.memset(v_aug[:, :, :, DK], 1.0)
        nc.scalar.activation(warm[:], wsrc[0:1, 0:DK], AF.Exp, scale=0.125)
        warm_slots = [
            pspool.tile([128, 512], fp32, tag="ps", name=f"warmps_{i}")
            for i in range(2)]
        warm_lt = [
            ltpool.tile([128, 512], fp32, tag="lt", name=f"warmlt_{i}")
            for i in range(4)]
        for i in range(8):
            wp = (warm_slots[i % 2][:, 0:512] if i % 4 < 2
                  else warm_lt[i % 4][:])
            nc.tensor.matmul(wp, wsrc[:, 0:128], wsrc[:, 0:512],
                             start=True, stop=True)
        for i in range(36):
            wp = (warm_slots[i % 2][:, 0:32] if i % 4 < 2
                  else warm_lt[i % 4][:, 0:32])
            nc.tensor.matmul(wp, wsrc[:, 0:128], wsrc[:, 0:32],
                             start=True, stop=True)
        # remaining x pieces; all 8 stay resident (q-proj of piece p runs
        # ~4 waves after its k-proj, so slots can't rotate)
        for p in range(1, 8):
            load_xq(p)

        # ---- projection pieces (as resumable quarter thunks) ----
        def qk_proj_thunks(p, w_sb, dst):
            """Four thunks of 2 chunks each for dst[:, p*512:(p+1)*512]."""
            hold = {}

            def quarter(i):
                def run():
                    if i == 0:
                        hold["ps"] = pspool.tile(
                            [128, 512], fp32, tag="ps",
                            name=f"pqk_{dst.tensor.name}_{p}")
                    ps = hold["ps"]
                    for c in range(2 * i, 2 * i + 2):
                        nc.tensor.matmul(ps[:], w_sb[:, c, :],
                                         xqs[p][:, c, :], start=(c == 0),
                                         stop=(c == 7))
                    if i == 3:
                        nc.vector.tensor_copy(
                            dst[:, p * 512:(p + 1) * 512], hold.pop("ps")[:])
                return run
            return [quarter(i) for i in range(4)]

        def v_block_thunk(tb):
            def run():
                p, i = tb // 4, tb % 4
                vps = pspool.tile([128, 128], fp32, tag="ps",
                                  name=f"vps_{tb}")
                for c in range(8):
                    nc.tensor.matmul(
                        vps[:], xqs[p][:, c, i * 128:(i + 1) * 128],
                        wv_sb[:, c, :], start=(c == 0), stop=(c == 7))
                nc.vector.tensor_copy(
                    v_aug[:, tb, :, 0:DK],
                    vps[:].rearrange("p (h d) -> p h d", h=2))
            return run

        # ---- startup projections: piece 0 of q and k, interleaved per
        # x-chunk so matmuls start as soon as each chunk DMA lands; the
        # two PSUM->SBUF copies run on different engines in parallel ----
        pq0 = pspool.tile([128, 512], fp32, tag="ps", name="pq0")
        pk0 = pspool.tile([128, 512], fp32, tag="ps", name="pk0")
        xq0 = xqs[0]
        for c in range(8):
            nc.tensor.matmul(pq0[:], wq_sb[:, c, :], xq0[:, c, :],
                             start=(c == 0), stop=(c == 7))
            nc.tensor.matmul(pk0[:], wk_sb[:, c, :], xq0[:, c, :],
                             start=(c == 0), stop=(c == 7))
        nc.scalar.copy(qT_sb[:, 0:512], pq0[:])
        # lt(0,0) needs only k block 0: give it its own small copy
        nc.vector.tensor_copy(kT_sb[:, 0:128], pk0[:, 0:128])
        nc.vector.tensor_copy(kT_sb[:, 128:512], pk0[:, 128:512])

        # deadline-ordered weave of the remaining projection work; each
        # entry is [deadline, pe_cost_ns, thunk]
        proj_q = deque()
        for p in range(1, 8):
            for j, t in enumerate(qk_proj_thunks(p, wk_sb, kT_sb)):
                proj_q.append([4 * p - 5 + j, 427, t])
        # v blocks 0..3 by deadline; the rest are emitted from inside the
        # wave-0 PV pops with a +4 lookahead, auto-tracking the drain rate
        v_done = set()

        def ensure_v(tb):
            if tb < NT and tb not in v_done:
                v_done.add(tb)
                v_block_thunk(tb)()

        n_pre_v = 4 if V_TRACK else NT
        for tb in range(n_pre_v):
            proj_q.append([LAG_W0 - 6 + tb, 427, lambda tb=tb: ensure_v(tb)])
        # q piece w must be fully copied before lt(w, 0) is emitted at
        # gi 32w-1: the last quarter lands at base+6 <= 32w-3
        for j, t in enumerate(qk_proj_thunks(1, wq_sb, qT_sb)):
            proj_q.append([(16 if Q_MID else 22) + 2 * j, 427, t])
        for w in range(2, NWV):
            for j, t in enumerate(qk_proj_thunks(w, wq_sb, qT_sb)):
                base = 32 * (w - 1) + (10 if Q_MID else 22)
                proj_q.append([base + 2 * j, 427, t])
        proj_q = deque(sorted(proj_q, key=lambda e: e[0]))

        # ---- attention machinery ----
        # pending: [floor, cost, kind, wave, thunk]; kind "pv" | "sc" | "fin".
        # Ordering gates (enforced at pop time, not just via floors):
        #   fin/sc(w) pop only after all pv(w) popped;
        #   pv(w+1) pops only after all sc(w) popped (acc slot handoff).
        pending = []
        live_pv = {w: 0 for w in range(NWV)}
        live_sc = {w: 0 for w in range(-1, NWV)}
        live_sc[-1] = 0
        state = {"gi": 0}
        lt_holder = {}

        def emit_lt(w, tb):
            # per-head lt tiles (1 PSUM bank each): each head's
            # lt -> exp -> slot-free chain ping-pongs independently across
            # 2 of the 4 slots, halving the latency the cadence must absorb
            s0 = w * 512
            lts = []
            for h in range(2):
                lt_h = ltpool.tile([128, 512], fp32, tag="lt",
                                   name=f"lt_{w}_{tb}_{h}")
                nc.tensor.matmul(
                    lt_h[:],
                    kT_sb[DK * h:DK * (h + 1), tb * 128:(tb + 1) * 128],
                    qT_sb[DK * h:DK * (h + 1), s0:s0 + 512],
                    start=True, stop=True,
                    tile_position=(DK * h, 0),
                )
                lts.append(lt_h)
            return lts

        def pv_thunk(w, tb, pt, accs):
            def run():
                if w == 0 and V_TRACK:
                    ensure_v(tb)
                    ensure_v(tb + 4)
                if tb == 0:
                    accs.append(accpool.tile([128, 2, 4, 128], fp32,
                                             tag="acc", name=f"acc_{w}"))
                acc = accs[0]
                p_all = (pt[:].bitcast(bf16) if pt.tensor.dtype == i16
                         else pt[:])
                for h in range(2):
                    for k in range(4):
                        # one accumulation group per PSUM zero-region (the
                        # 2KB bank holding all 4 k-slices of head h): start
                        # marks the whole bank pending-zero, so only the
                        # first slice may start and only the last may stop;
                        # the other tb==0 writes zero-on-first-touch.
                        nc.tensor.matmul(
                            acc[:, h, k, 0:DK + 1],
                            p_all[:, h * 512 + k * 128:h * 512 + (k + 1) * 128],
                            v_aug[:, tb, h, :],
                            start=(tb == 0 and k == 0),
                            stop=(tb == NT - 1 and k == 3),
                            skip_group_check=(k != 0),
                        )
            return run

        def finalize_thunks(w, accs):
            rden = dpool.tile([128, 2, 4], fp32, tag="rden", name=f"rden_{w}")
            attn_all = attnpool.tile([128, 2, 4, DK], bf16, tag="attn",
                                     name=f"attn_{w}")
            atT = {}

            def recip():
                nc.vector.reciprocal(rden[:], accs[0][:, :, :, DK])

            def scale_ks(k0, k1):
                # one DVE op normalizes all (h, k0:k1) blocks: rden broadcast
                # along dk via a stride-0 free dim
                nc.vector.tensor_tensor(
                    attn_all[:, :, k0:k1, :], accs[0][:, :, k0:k1, 0:DK],
                    rden[:, :, k0:k1, None].to_broadcast([128, 2, k1 - k0, DK]),
                    ALU.mult)

            def scale_act(k):
                # last wave: late blocks on the otherwise-idle ACT so their
                # transposes unblock without waiting out DVE's queue
                for h in range(2):
                    nc.scalar.activation(attn_all[:, h, k, :],
                                         accs[0][:, h, k, 0:DK],
                                         AF.Copy, scale=rden[:, h, k:k + 1])

            last = w == NWV - 1

            def transp(k):
                # both heads transpose into one [128,128] psum tile (disjoint
                # partition ranges -> no zero-region conflict), one copy out
                atT[k] = atTpool.tile([128, 128], bf16, tag="atT",
                                      name=f"atT_{w}_{k}")
                tps = pspool.tile([128, 128], bf16, tag="ps",
                                  name=f"tps_{w}_{k}")
                for h in range(2):
                    nc.tensor.transpose(tps[DK * h:DK * (h + 1), :],
                                        attn_all[:, h, k, :], id_sb[:])
                nc.vector.tensor_copy(atT[k][:], tps[:])

            def yblock(k):
                # one combined [128,1024] output + ONE y DMA per s-block:
                # HWDGE serializes issues at ~625ns, so halving the DMA
                # count shortens the end-of-kernel trickle
                b = w * 4 + k
                yo = yopool.tile([128, 1024], bf16, tag="yo",
                                 name=f"yo_{b}")
                if last:
                    # the lt slots are dead after the final exp and the acc
                    # banks after the scales: draw per-half yp banks from
                    # them so the tail pipelines 4+ deep, and split the
                    # copies across the idle ACT and DVE
                    if k == 3:
                        # acc banks free only after every scale has read
                        # them — give them to the LAST block
                        ypt = accpool.tile([128, 2, 512], fp32, tag="acc",
                                           name=f"yp_{b}")
                        yps = [ypt[:, jc, :] for jc in range(2)]
                    else:
                        yps = [ltpool.tile([128, 512], fp32, tag="lt",
                                           name=f"yp_{b}_{jc}")[:]
                               for jc in range(2)]
                    for jc in range(2):
                        nc.tensor.matmul(
                            yps[jc], atT[k][:],
                            wo_sb[:, jc * 512:(jc + 1) * 512],
                            start=True, stop=True)
                    nc.scalar.copy(yo[:, 0:512], yps[0])
                    nc.vector.tensor_copy(yo[:, 512:1024], yps[1])
                else:
                    for jc in range(2):
                        yp = pspool.tile([128, 512], fp32, tag="ps",
                                         name=f"yp_{b}_{jc}")
                        nc.tensor.matmul(
                            yp[:], atT[k][:],
                            wo_sb[:, jc * 512:(jc + 1) * 512],
                            start=True, stop=True)
                        if jc == 0 and YO_ACT:
                            nc.scalar.copy(yo[:, 0:512], yp[:])
                        else:
                            nc.vector.tensor_copy(
                                yo[:, jc * 512:(jc + 1) * 512], yp[:])
                nc.sync.dma_start(y[b * 128:(b + 1) * 128, :], yo[:])

            # (floor_offset, pe_cost_ns, thunk): scales all run first (they
            # are acc's only readers, so the next wave's PV start unblocks
            # early); transposes and output blocks then trickle so the
            # finalize never oversubscribes an iteration's PE slack
            if last:
                thunks = [(1, 10, "sc", "dve", recip)]
                thunks.append((2, 10, "sc", "dve", lambda: scale_ks(0, 2)))
                thunks.append((3, 10, "sc", "act", lambda: scale_act(2)))
                thunks.append((4, 10, "sc", "act", lambda: scale_act(3)))
            else:
                # floors picked so each DVE/ACT-touching thunk pops at an
                # iteration whose current AND next exp are on the other
                # engine (a popped copy queued just before an exp delays
                # the lt slot free and stalls the PE)
                thunks = [(RECIP_F, 10, "sc", "dve", recip)]
                thunks.append((RECIP_F + 1, 10, "sc", "dve",
                               lambda: scale_ks(0, 4)))
            for k in range(4):
                if last:
                    # tail: engines are draining — emit as soon as inputs
                    # allow so the final blocks pipeline deep
                    thunks.append((3 + 4 * k, 110, "fin", "dve",
                                   lambda k=k: transp(k)))
                    thunks.append((4 + 4 * k, 430, "fin", None,
                                   lambda k=k: yblock(k)))
                else:
                    thunks.append((TR_F[k], 110, "fin", "dve",
                                   lambda k=k: transp(k)))
                    thunks.append((YB_F[k], 430, "fin", "dve",
                                   lambda k=k: yblock(k)))
            return thunks

        def emit_iter(w, tb, accs):
            gi = state["gi"]
            lag = {0: LAG_W0, 1: LAG_W1, NWV - 1: LAG_LAST}.get(w, LAG)
            if w >= 2 and w != NWV - 1:
                # smooth the wave-boundary acc handoff (PVlast(w-1) -> recip
                # -> scale -> PV0(w), accpool bufs=1): give the first PVs of
                # the wave extra lag so the serial chain hides under lt/proj
                # work instead of stalling the PE
                lag = max(LAG, LAG_RAMP - tb)
            lt = lt_holder.pop("lt")
            dve_tbs = DVE_TBS_W0 if w == 0 else DVE_TBS
            # exp in per-head halves: the lt slot's h0 half frees one
            # exp-half earlier, cutting the lt->exp->slot-free round trip
            # below the PE iteration time (the slot ping-pong is only 2 deep;
            # PSUM can't fit 3) — subtile deps let lt(i+2)'s h0 matmul start
            # as soon as exp(i)'s h0 half completes
            if tb in dve_tbs:
                # Schraudolph exp on DVE: affine into int16, bitcast bf16
                pt = ptpool.tile([128, 1024], i16, tag="pt",
                                 name=f"pt_{w}_{tb}")
                for hh in range(2):
                    nc.vector.tensor_scalar(
                        pt[:, hh * 512:(hh + 1) * 512],
                        lt[hh][:], SCH_A, SCH_B,
                        ALU.mult, ALU.add)
            else:
                pt = ptpool.tile([128, 1024], bf16, tag="pt",
                                 name=f"pt_{w}_{tb}")
                for hh in range(2):
                    nc.scalar.activation(pt[:, hh * 512:(hh + 1) * 512],
                                         lt[hh][:],
                                         AF.Exp, scale=0.125)
            if tb + 1 < NT:
                lt_holder["lt"] = emit_lt(w, tb + 1)
            elif w + 1 < NWV:
                lt_holder["lt"] = emit_lt(w + 1, 0)
            live_pv[w] += 1
            cur_eng = "dve" if tb in dve_tbs else "act"
            nxt_eng = "dve" if (tb + 1) in dve_tbs else "act"
            pending.append([gi + lag, 644 if (w == 0 and V_TRACK) else 217,
                            "pv", None, w, pv_thunk(w, tb, pt, accs)])
            # Pop READY items anywhere in the list (a far-future finalize
            # floor must not head-block the PV stream), but cap the popped
            # PE-ns per iteration, and defer evacuation thunks whose engine
            # just received this iteration's exp (a copy queued between exp
            # halves delays the lt slot free and stalls the PE).
            budget = PE_BUDGET
            i = 0
            while i < len(pending):
                floor, cost, kind, eng, wv, t = pending[i]
                ok = floor <= gi and (cost <= budget or floor <= gi - 12)
                if ok and eng == cur_eng and floor > gi - 6:
                    ok = False
                if ok and kind == "pv":
                    ok = live_sc[wv - 1] == 0
                elif ok:
                    ok = live_pv[wv] == 0
                if ok:
                    pending.pop(i)
                    t()
                    budget -= cost
                    if kind == "pv":
                        live_pv[wv] -= 1
                    elif kind == "sc":
                        live_sc[wv] -= 1
                else:
                    i += 1
            # projection deadlines are HARD (logits read qT/kT at fixed
            # iterations): pop regardless of remaining budget
            pops = 0
            while proj_q and proj_q[0][0] <= gi and pops < 2:
                proj_q.popleft()[2]()
                pops += 1
            state["gi"] = gi + 1

        # ---- main loop ----
        lt_holder["lt"] = emit_lt(0, 0)
        for w in range(NWV):
            accs = []
            for tb in range(NT):
                emit_iter(w, tb, accs)
            for off, cost, kind, eng, t in finalize_thunks(w, accs):
                if kind == "sc":
                    live_sc[w] += 1
                pending.append([state["gi"] + LAG - 2 + off, cost, kind,
                                eng, w, t])
        while proj_q:
            proj_q.popleft()[2]()
        # final drain: keep list order within a wave; gates are satisfied
        # by construction (pv entries precede sc precede fin per wave)
        for e in pending:
            e[5]()

    _split_multi_waits(nc, mybir)
    nc.finalize()
    return nc


def _get_nc():
    if "nc" not in _NC_CACHE:
        _NC_CACHE["nc"] = _build_nc()
    return _NC_CACHE["nc"]


def _relay(wT):
    """[1024 d, 128 m] -> [p, c*m] with wT[c*128+p, m] at [p, c, m]: every
    DMA descriptor becomes a contiguous 2KB run."""
    return np.ascontiguousarray(
        wT.reshape(8, 128, HD).transpose(1, 0, 2).reshape(128, 8 * HD))


def _in_maps(x, Wq, Wk, Wv, Wo):
    import ml_dtypes
    bf16 = ml_dtypes.bfloat16
    xT = np.ascontiguousarray(np.asarray(x, np.float32).T).astype(bf16)
    ident = np.eye(128, dtype=np.float32).astype(bf16)
    maps = []
    for c in range(NCORES):
        sl = slice(HD * c, HD * (c + 1))
        maps.append(dict(
            xT=xT,
            wqT=_relay(np.asarray(Wq)[sl, :].T.astype(bf16)),
            wkT=_relay(np.asarray(Wk)[sl, :].T.astype(bf16)),
            wvT=_relay(np.asarray(Wv)[sl, :].T.astype(bf16)),
            woT=np.ascontiguousarray(np.asarray(Wo)[:, sl].T).astype(bf16),
            ident=ident,
        ))
    return maps


def kernel(x, Wq, Wk, Wv, Wo):
    from concourse.bass_utils import run_bass_kernel_spmd

    x = np.asarray(x, dtype=np.float32)
    nc = _get_nc()
    res = run_bass_kernel_spmd(nc, _in_maps(x, Wq, Wk, Wv, Wo),
                               list(range(NCORES)))
    out = np.zeros((S, D), np.float32)
    for rr in res.results:
        out += np.asarray(rr["y"], dtype=np.float32)
    return out

